# revision 21
# baseline (speedup 1.0000x reference)
"""CSAEncoder Trainium2 kernel: 3-branch cross-attention + concat DoubleConv.

Sharding (8 cores): 2 batch groups x 4 tensor ranks.
Core c: batch b = c // 4, rank g = c % 4.
  - Attention: core computes heads [4g, 4g+4) of all 3 branches for batch b
    (a contiguous 128-channel slab of each branch's output).
  - conv1 computed as partial sums over the core's local 384 input channels
    for ALL 512 output channels; per-branch bf16 AllReduce(add) within the
    4-core batch group (branches 0/1 overlap later attention; branch 2 is
    split into two channel chunks pipelined against h1+conv2).
  - conv2 computed locally: full 512-channel contraction, only the core's own
    128 output channels. No further collective.
Host assembles the full (2, 512, 32, 32) output from the 8 per-core slabs.

v2 changes vs v1:
  - BN bias of each attention branch folded into the v/o projection bias
    host-side (y + b*denom = sum_k (u+b) p), so the post-softmax division is
    a single tensor_mul per head.
  - Softmax denominators: reciprocal_approx_fast (DVE) + gpsimd
    partition_broadcast (SBUF->SBUF) instead of full-precision DVE
    reciprocal + DMA roundtrip through DRAM.
  - y-matmul chains column-packed in pairs (tile_position (0,0)/(0,64)).
  - Per-branch AllReduce in bf16 instead of one fp32 AllReduce at the end.
  - Consolidated input DMAs.
"""

import os
import sys

import ml_dtypes
import numpy as np

for _p in ("/opt/trn_rl_repo",):
    if _p not in sys.path and os.path.isdir(_p):
        sys.path.insert(0, _p)

import concourse.bass as bass
import concourse.mybir as mybir
import concourse.tile as tile
from concourse import bacc
from concourse.bass_utils import run_bass_kernel_spmd

F32 = mybir.dt.float32
BF16 = mybir.dt.bfloat16
AF = mybir.ActivationFunctionType
B, C, H, W, HEADS = 2, 512, 32, 32, 16
D = C // HEADS            # 32
S = H * W                 # 1024
EPS = 1e-5
ISQD = 1.0 / np.sqrt(D)   # folded into the exp activation
NCORES = 8
GROUPS = [[0, 1, 2, 3], [4, 5, 6, 7]]
HP = W + 2                # padded row stride (34)


def build_nc():
    nc = bacc.Bacc(None, target_bir_lowering=False)

    # ---- per-core external inputs -------------------------------------
    x4_d = nc.declare_dram_parameter("x4", [4, 128, S], BF16, isOutput=False)
    oth_d = nc.declare_dram_parameter("oth", [2, 4, 128, S], BF16, isOutput=False)
    wqT_d = nc.declare_dram_parameter("wqT", [3, 4, 128, 128], BF16, isOutput=False)
    wkT_d = nc.declare_dram_parameter("wkT", [3, 4, 128, 128], BF16, isOutput=False)
    wvoT_d = nc.declare_dram_parameter("wvoT", [4, 128, 384], BF16, isOutput=False)
    dvec_d = nc.declare_dram_parameter("dvec", [128, 6], F32, isOutput=False)
    wobv_d = nc.declare_dram_parameter("wobv", [1, 384], F32, isOutput=False)
    c1wT_d = nc.declare_dram_parameter("c1wT", [3, 4, 128, 9, 128], BF16, isOutput=False)
    c2wT_d = nc.declare_dram_parameter("c2wT", [4, 128, 9, 128], BF16, isOutput=False)
    avec_d = nc.declare_dram_parameter("avec", [128, 10], F32, isOutput=False)
    out_d = nc.declare_dram_parameter("out", [128, S], F32, isOutput=True)

    with tile.TileContext(nc) as tc:
        import contextlib

        ctx = contextlib.ExitStack()
        with ctx:
            const = ctx.enter_context(tc.tile_pool(name="const", bufs=1))
            kq = ctx.enter_context(tc.tile_pool(name="kq", bufs=1))
            xtp = ctx.enter_context(tc.tile_pool(name="xtp", bufs=1))
            stg = ctx.enter_context(tc.tile_pool(name="stg", bufs=1))
            brp = ctx.enter_context(tc.tile_pool(name="brp", bufs=2))
            rcp = ctx.enter_context(tc.tile_pool(name="rcp", bufs=2))
            scps = ctx.enter_context(tc.tile_pool(name="scps", bufs=2, space="PSUM"))
            yps = ctx.enter_context(tc.tile_pool(name="yps", bufs=1, space="PSUM"))
            smps = ctx.enter_context(tc.tile_pool(name="smps", bufs=2, space="PSUM"))
            dram = ctx.enter_context(tc.tile_pool(name="dram", bufs=1, space="DRAM"))
            dramw = ctx.enter_context(tc.tile_pool(name="dramw", bufs=4, space="DRAM"))

            # ---- activations + branch-0 weights first (DMA priority) -----
            x_sb = const.tile([128, 4, S], BF16)
            nc.sync.dma_start(out=x_sb, in_=x4_d[:].rearrange("k p s -> p k s"))
            wq_sb = const.tile([128, 3, 4, 128], BF16)
            wk_sb = const.tile([128, 3, 4, 128], BF16)
            nc.sync.dma_start(out=wk_sb, in_=wkT_d[:].rearrange("i k p f -> p i k f"))
            nc.sync.dma_start(out=wq_sb, in_=wqT_d[:].rearrange("i k p f -> p i k f"))
            othp = ctx.enter_context(tc.tile_pool(name="othp", bufs=1))
            oth1 = othp.tile([128, 4, S], BF16, name="oth")
            nc.sync.dma_start(out=oth1, in_=oth_d[0].rearrange("k p s -> p k s"))
            wvo_sb = const.tile([128, 4, 384], BF16)
            nc.sync.dma_start(out=wvo_sb, in_=wvoT_d[:].rearrange("k p f -> p k f"))

            # Small consts: DMA to staging, then re-own on the consuming
            # engine (DVE / ACT) so consumers need no cross-engine const wait.
            dvec_st = const.tile([128, 6], F32)
            nc.gpsimd.dma_start(out=dvec_st, in_=dvec_d[:])
            wobv_st = const.tile([128, 384], F32)
            nc.gpsimd.dma_start(out=wobv_st, in_=wobv_d[:].partition_broadcast(128))
            avec_st = const.tile([128, 10], F32)
            nc.gpsimd.dma_start(out=avec_st, in_=avec_d[:])
            dvec = const.tile([128, 6], F32)
            nc.vector.tensor_copy(dvec, dvec_st)
            wobv_sb = const.tile([128, 384], F32)
            nc.vector.tensor_copy(wobv_sb, wobv_st)
            avec = const.tile([128, 10], F32)
            nc.scalar.activation(out=avec, in_=avec_st, func=AF.Copy)
            bqv_sb = dvec[:, 0:3]
            bkv_sb = dvec[:, 3:6]

            # xt (attention output) slabs + h1 slabs, zero-padded 34x34
            xt_sl = []
            for i in range(3):
                t = xtp.tile([128, HP, HP], BF16, name=f"xt{i}")
                nc.vector.memset(t, 0.0)
                xt_sl.append(t)
            h1_sl = []
            for k in range(4):
                t = xtp.tile([128, HP, HP], BF16, name=f"h1{k}")
                nc.vector.memset(t, 0.0)
                h1_sl.append(t)
            # per-branch conv1 partials (f32 staging for the AllReduce);
            # 2 rotating slots — branch i is shipped before i+1 is written
            brst = {}

            def get_brst(i):
                if i not in brst:
                    brst[i] = brp.tile([128, 4, S], F32, name="brst")
                return brst[i]

            # Semaphore warmers: absorb const-DMA + memset waits into each
            # engine's observed clock so later compute ops need <=1 wait.
            warm = const.tile([128, 1], F32)
            nc.vector.tensor_copy(warm, dvec[:, 0:1])
            warm2 = const.tile([128, 1], F32)
            nc.scalar.activation(out=warm2, in_=warm, func=AF.Copy)

            # k/q per branch (with biases added), uT tiles
            k_sb = kq.tile([128, 3, S], BF16)
            q_sb = kq.tile([128, 3, S], BF16)
            uT = [kq.tile([128, 3, 4, 33], BF16, name=f"uT{t}") for t in range(8)]

            # ---- projections (k0/q0 upfront; rest fill branch-0 slots) ---
            # x_prev reuses x_next's slot once q_proj(0) has consumed it
            qsrc = [oth1, x_sb, None]

            def load_oth2():
                oth2 = othp.tile([128, 4, S], BF16, name="oth")
                nc.sync.dma_start(out=oth2, in_=oth_d[1].rearrange("k p s -> p k s"))
                qsrc[2] = oth2

            def k_proj(i):
                k_ps = scps.tile([128, S], F32, name="kq_ps", tag="sc")
                for s in range(2):
                    for ks in range(4):
                        nc.tensor.matmul(
                            k_ps[:, 512 * s : 512 * (s + 1)],
                            lhsT=wk_sb[:, i, ks, :],
                            rhs=x_sb[:, ks, 512 * s : 512 * (s + 1)],
                            start=(ks == 0),
                            stop=(ks == 3),
                        )
                nc.vector.tensor_scalar_add(k_sb[:, i, :], k_ps, bkv_sb[:, i : i + 1])

            def q_proj(i):
                q_ps = scps.tile([128, S], F32, name="kq_ps2", tag="sc")
                for s in range(2):
                    for ks in range(4):
                        nc.tensor.matmul(
                            q_ps[:, 512 * s : 512 * (s + 1)],
                            lhsT=wq_sb[:, i, ks, :],
                            rhs=qsrc[i][:, ks, 512 * s : 512 * (s + 1)],
                            start=(ks == 0),
                            stop=(ks == 3),
                        )
                nc.vector.tensor_scalar_add(q_sb[:, i, :], q_ps, bqv_sb[:, i : i + 1])

            def u_proj(t):
                u_ps = smps.tile([128, 384], F32, name="u_ps", tag="sm")
                for ks in range(4):
                    nc.tensor.matmul(
                        u_ps,
                        lhsT=x_sb[:, ks, 128 * t : 128 * (t + 1)],
                        rhs=wvo_sb[:, ks, :],
                        start=(ks == 0),
                        stop=(ks == 3),
                    )
                nc.vector.memset(uT[t][:, :, :, 32:33], 1.0)
                # wobv has the attention-BN bias folded in host-side
                nc.vector.tensor_add(
                    uT[t][:, :, :, 0:32],
                    u_ps.rearrange("p (i h d) -> p i h d", i=3, h=4),
                    wobv_sb.rearrange("p (i h d) -> p i h d", i=3, h=4),
                )

            k_proj(0)
            q_proj(0)
            load_oth2()

            # ---- conv weights (emitted after proj psum freed) ------------
            convw = ctx.enter_context(tc.tile_pool(name="convw", bufs=1))
            pt = ctx.enter_context(tc.tile_pool(name="pt", bufs=16))
            c1w_sb = [
                [convw.tile([128, 9, 128], BF16, name=f"c1w{i}_{m}") for m in range(4)]
                for i in range(3)
            ]
            for i in range(3):
                for m in range(4):
                    nc.sync.dma_start(out=c1w_sb[i][m], in_=c1wT_d[i, m])
            c2w_sb = [convw.tile([128, 9, 128], BF16, name=f"c2w{k}") for k in range(4)]
            for k in range(4):
                nc.sync.dma_start(out=c2w_sb[k], in_=c2wT_d[k])

            def conv1_block(i, m, n):
                """Partial conv1 for xt slab i, out m-tile, spatial half n,
                written (bf16) into brst[i]."""
                ps = smps.tile([128, 512], F32, name="cv", tag="sm")
                for dy in range(3):
                    for dx in range(3):
                        nc.tensor.matmul(
                            ps,
                            lhsT=c1w_sb[i][m][:, dy * 3 + dx, :],
                            rhs=xt_sl[i][:, 16 * n + dy : 16 * n + dy + 16, dx : dx + 32],
                            start=(dy == 0 and dx == 0),
                            stop=(dy == 2 and dx == 2),
                        )
                nc.vector.tensor_copy(get_brst(i)[:, m, 512 * n : 512 * (n + 1)], ps)
                if n == 1:
                    if i < 2:
                        dst = partial[i][128 * m : 128 * (m + 1), :]
                    else:
                        dst = partial2[m // 2][128 * (m % 2) : 128 * (m % 2) + 128, :]
                    nc.sync.dma_start(out=dst, in_=get_brst(i)[:, m, :])

            def attention(i, pr, filler):
                """Heads (2pr, 2pr+1) of branch i.  `filler` is a list of
                thunks (conv1 blocks / projections) sprinkled between the
                per-t score groups to keep PE dense while ACT grinds exps."""
                heads = (2 * pr, 2 * pr + 1)
                pts = {}
                fi = 0
                for t in range(8):
                    for h in heads:
                        sc = scps.tile([128, S], F32, name="sc", tag="sc")
                        p0 = 32 * h
                        for s in range(2):
                            nc.tensor.matmul(
                                sc[:, 512 * s : 512 * (s + 1)],
                                lhsT=k_sb[p0 : p0 + 32, i, 128 * t : 128 * (t + 1)],
                                rhs=q_sb[p0 : p0 + 32, i, 512 * s : 512 * (s + 1)],
                                start=True,
                                stop=True,
                                tile_position=(p0, 0),
                            )
                        ptt = pt.tile([128, S], BF16, name="ptt")
                        nc.scalar.activation(
                            out=ptt, in_=sc, func=AF.Exp, scale=float(ISQD)
                        )
                        pts[(h, t)] = ptt
                    while fi < len(filler) * (t + 1) // 8:
                        filler[fi]()
                        fi += 1
                # y chains: per head, the two query-half chains target the
                # two zero regions of one [33, S] psum tile sequentially
                for h in heads:
                    y2 = yps.tile([33, S], F32, name="y2", tag="y")
                    for s in range(2):
                        for t in range(8):
                            nc.tensor.matmul(
                                y2[:, 512 * s : 512 * (s + 1)],
                                lhsT=uT[t][:, i, h, :],
                                rhs=pts[(h, t)][:, 512 * s : 512 * (s + 1)],
                                start=(t == 0),
                                stop=(t == 7),
                            )
                    p0 = 32 * h
                    rc = rcp.tile([1, S], F32, name="rc")
                    nc.vector.reciprocal(rc, y2[32:33, :])
                    rcd = dramw.tile([1, S], F32, name="rcd", tag="rcd")
                    nc.gpsimd.dma_start(out=rcd, in_=rc)
                    rcb = rcp.tile([32, S], F32, name="rcb")
                    nc.gpsimd.dma_start(out=rcb, in_=rcd[:].partition_broadcast(32))
                    nc.vector.tensor_mul(
                        xt_sl[i][p0 : p0 + 32, 1:33, 1:33],
                        y2[0:32, :].rearrange("p (a b) -> p a b", b=32),
                        rcb.rearrange("p (a b) -> p a b", b=32),
                    )
                while fi < len(filler):
                    filler[fi]()
                    fi += 1

            # ---- collectives ---------------------------------------------
            # branch 0/1: one [512, S] bf16 AllReduce each, overlapped with
            # the next branch's attention.  branch 2: two [256, S] chunks
            # (m01 / m23) pipelined against h1+conv2.
            partial = [dram.tile([512, S], F32, name=f"part{i}") for i in range(2)]
            art = [dram.tile([512, S], F32, name=f"art{i}") for i in range(2)]
            partial2 = [dram.tile([256, S], F32, name=f"part2{a}") for a in range(2)]
            art2 = [dram.tile([256, S], F32, name=f"art2{a}") for a in range(2)]

            def ar_branch(i):
                nc.gpsimd.collective_compute(
                    "AllReduce",
                    mybir.AluOpType.add,
                    replica_groups=GROUPS,
                    ins=[partial[i][:]],
                    outs=[art[i][:]],
                )

            def ar2_chunk(a):
                nc.gpsimd.collective_compute(
                    "AllReduce",
                    mybir.AluOpType.add,
                    replica_groups=GROUPS,
                    ins=[partial2[a][:]],
                    outs=[art2[a][:]],
                )

            def conv1_and_ship(i):
                return [
                    (lambda m=m, n=n: conv1_block(i, m, n))
                    for m in range(4)
                    for n in range(2)
                ]

            # ---- phase A: attention with projections/conv1 interleaved ---
            attention(0, 0, [lambda t=t: u_proj(t) for t in range(8)])
            attention(0, 1, [lambda: k_proj(1), lambda: q_proj(1),
                             lambda: k_proj(2), lambda: q_proj(2)])
            f0 = conv1_and_ship(0)
            attention(1, 0, f0[:4])
            attention(1, 1, f0[4:] + [lambda: ar_branch(0)])
            f1 = conv1_and_ship(1)
            attention(2, 0, f1[:4])
            attention(2, 1, f1[4:] + [lambda: ar_branch(1)])
            # branch 2 conv1 + chunked AR at the end
            for m in range(4):
                for n in range(2):
                    conv1_block(2, m, n)
                if m == 1:
                    ar2_chunk(0)
            ar2_chunk(1)

            # ---- phase C: combine, BN1+relu, conv2, BN2+relu, out --------
            arr01 = stg.tile([128, 4, S], BF16, name="arr01", bufs=1)
            arrt = [stg.tile([128, 2, S], F32, name=f"arrt{j}", bufs=1) for j in range(2)]
            # art0 + art1 (during late attention / AR2 flight)
            nc.gpsimd.dma_start(
                out=arr01, in_=art[0][:].rearrange("(m p) s -> p m s", p=128)
            )
            nc.gpsimd.dma_start(
                out=arrt[0],
                in_=art[1][:].rearrange("(m p) s -> p m s", p=128)[:, 0:2, :],
            )
            nc.gpsimd.dma_start(
                out=arrt[1],
                in_=art[1][:].rearrange("(m p) s -> p m s", p=128)[:, 2:4, :],
            )
            nc.vector.tensor_add(
                arr01[:, 0:2, :], arr01[:, 0:2, :], arrt[0][:, 0:2, :]
            )
            nc.vector.tensor_add(
                arr01[:, 2:4, :], arr01[:, 2:4, :], arrt[1][:, 0:2, :]
            )

            oout = stg.tile([128, S], F32, name="oout", bufs=1)
            ps2 = [smps.tile([128, 512], F32, name=f"cv2_{n}", tag="sm") for n in range(2)]

            def h1_chunk(a):
                """arr01[m01/m23] + art2 chunk -> BN1+relu -> h1 slabs."""
                nc.gpsimd.dma_start(
                    out=arrt[0],
                    in_=art2[a][:].rearrange("(m p) s -> p m s", p=128),
                )
                src = arrt[0]
                nc.vector.tensor_add(
                    src[:, 0:2, :], src[:, 0:2, :], arr01[:, 2 * a : 2 * a + 2, :]
                )
                for j, k in enumerate((2 * a, 2 * a + 1)):
                    nc.scalar.activation(
                        out=h1_sl[k][:, 1:33, 1:33],
                        in_=src[:, j, :].rearrange("p (a b) -> p a b", b=32),
                        func=AF.Relu,
                        bias=avec[:, 4 + k : 5 + k],
                        scale=avec[:, k : k + 1],
                    )

            def conv2_half(a):
                # accumulate k-slabs 2a, 2a+1 into both spatial halves
                for n in range(2):
                    for k in (2 * a, 2 * a + 1):
                        for dy in range(3):
                            for dx in range(3):
                                nc.tensor.matmul(
                                    ps2[n],
                                    lhsT=c2w_sb[k][:, dy * 3 + dx, :],
                                    rhs=h1_sl[k][
                                        :, 16 * n + dy : 16 * n + dy + 16, dx : dx + 32
                                    ],
                                    start=(k == 0 and dy == 0 and dx == 0),
                                    stop=(k == 3 and dy == 2 and dx == 2),
                                )

            h1_chunk(0)
            conv2_half(0)   # overlaps AR2 chunk 1
            h1_chunk(1)
            conv2_half(1)
            for n in range(2):
                nc.scalar.activation(
                    out=oout[:, 512 * n : 512 * (n + 1)],
                    in_=ps2[n],
                    func=AF.Relu,
                    bias=avec[:, 9:10],
                    scale=avec[:, 8:9],
                )
                nc.sync.dma_start(
                    out=out_d[:, 512 * n : 512 * (n + 1)],
                    in_=oout[:, 512 * n : 512 * (n + 1)],
                )

    nc.finalize()
    return nc


def _f(x):
    return np.ascontiguousarray(x, dtype=np.float32)


def _bf(x):
    return np.ascontiguousarray(np.asarray(x, dtype=np.float32).astype(ml_dtypes.bfloat16))


def prepare_core_inputs(inp):
    """Build the 8 per-core input dicts from the full-problem inputs."""
    inp = {k: np.asarray(v, dtype=np.float64) for k, v in inp.items()}
    x = inp["x"].reshape(B, C, S)
    xp = inp["x_prev"].reshape(B, C, S)
    xn = inp["x_next"].reshape(B, C, S)

    bn1s_full = inp["bn1g"] / np.sqrt(inp["bn1v"] + EPS)
    bn1b_full = inp["bn1b"] - inp["bn1m"] * bn1s_full
    bn2s_full = inp["bn2g"] / np.sqrt(inp["bn2v"] + EPS)
    bn2b_full = inp["bn2b"] - inp["bn2m"] * bn2s_full

    per_g = []
    for g in range(4):
        sl = slice(128 * g, 128 * (g + 1))
        wqT = np.stack(
            [
                np.stack([inp["Wq"][i][sl, 128 * k : 128 * (k + 1)].T for k in range(4)])
                for i in range(3)
            ]
        )
        wkT = np.stack(
            [
                np.stack([inp["Wk"][i][sl, 128 * k : 128 * (k + 1)].T for k in range(4)])
                for i in range(3)
            ]
        )
        bqv = np.stack([inp["bq"][i][sl] for i in range(3)], axis=1)
        bkv = np.stack([inp["bk"][i][sl] for i in range(3)], axis=1)

        att_s = np.stack(
            [inp["bng"][i][sl] / np.sqrt(inp["bnv"][i][sl] + EPS) for i in range(3)]
        )  # (3,128)
        xtb = np.stack(
            [
                inp["bnb"][i][sl] + (inp["bo"][i][sl] - inp["bnm"][i][sl]) * att_s[i]
                for i in range(3)
            ]
        )  # (3,128)

        wvo_rows = []
        wobv_row = []
        for i in range(3):
            for hl in range(4):
                hg = 4 * g + hl
                wv_h = inp["Wv"][i][32 * hg : 32 * (hg + 1), :]  # (32, 512)
                bv_h = inp["bv"][i][32 * hg : 32 * (hg + 1)]
                wo_h = inp["Wo"][i, hg]  # (32, 32)
                sc = att_s[i][32 * hl : 32 * (hl + 1)]  # (32,)
                wvo_rows.append(sc[:, None] * (wo_h @ wv_h))
                # fold the (BN-scaled) output bias + BN bias into the u bias:
                # y/denom + xtb == sum_k (u + xtb) p_k / denom
                wobv_row.append(sc * (wo_h @ bv_h) + xtb[i][32 * hl : 32 * (hl + 1)])
        wvo_all = np.concatenate(wvo_rows, axis=0)  # (384, 512)
        wobv = np.concatenate(wobv_row)[None, :]  # (1, 384)
        wvoT = np.stack([wvo_all[:, 128 * k : 128 * (k + 1)].T for k in range(4)])

        c1wT = np.stack(
            [
                np.stack(
                    [
                        inp["c1w"][
                            128 * m : 128 * (m + 1),
                            512 * i + 128 * g : 512 * i + 128 * (g + 1),
                        ]
                        .transpose(1, 2, 3, 0)
                        .reshape(128, 9, 128)
                        for m in range(4)
                    ]
                )
                for i in range(3)
            ]
        )
        c2wT = np.stack(
            [
                inp["c2w"][sl, 128 * k : 128 * (k + 1)]
                .transpose(1, 2, 3, 0)
                .reshape(128, 9, 128)
                for k in range(4)
            ]
        )
        avec = np.concatenate(
            [
                bn1s_full.reshape(4, 128).T,
                bn1b_full.reshape(4, 128).T,
                bn2s_full[sl][:, None],
                bn2b_full[sl][:, None],
            ],
            axis=1,
        )  # (128, 10)

        per_g.append(
            dict(
                wqT=_bf(wqT), wkT=_bf(wkT), wvoT=_bf(wvoT),
                wobv=_f(wobv), c1wT=_bf(c1wT), c2wT=_bf(c2wT),
                dvec=_f(np.concatenate([bqv, bkv], axis=1)),
                avec=_f(avec),
            )
        )

    in_maps = []
    for c in range(NCORES):
        b, g = c // 4, c % 4
        d = dict(per_g[g])
        d["x4"] = _bf(x[b].reshape(4, 128, S))
        d["oth"] = _bf(np.stack([xn[b].reshape(4, 128, S), xp[b].reshape(4, 128, S)]))
        in_maps.append(d)
    return in_maps


_NC_CACHE = {}


def get_nc():
    if "nc" not in _NC_CACHE:
        _NC_CACHE["nc"] = build_nc()
    return _NC_CACHE["nc"]


def assemble(results):
    out = np.zeros((B, C, H, W), dtype=np.float32)
    for c in range(NCORES):
        b, g = c // 4, c % 4
        out[b, 128 * g : 128 * (g + 1)] = results[c]["out"].reshape(128, H, W)
    return out


def kernel(**inputs):
    nc = get_nc()
    in_maps = prepare_core_inputs(inputs)
    res = run_bass_kernel_spmd(nc, in_maps, list(range(NCORES)))
    return assemble(res.results)


# revision 25
# speedup vs baseline: 1.1059x; 1.1059x over previous
"""CSAEncoder Trainium2 kernel: 3-branch cross-attention + concat DoubleConv.

Sharding (8 cores): 2 batch groups x 4 tensor ranks.
Core c: batch b = c // 4, rank g = c % 4.
  - Attention: core computes heads [4g, 4g+4) of all 3 branches for batch b
    (a contiguous 128-channel slab of each branch's output).
  - conv1 computed as partial sums over the core's local 384 input channels
    for ALL 512 output channels; per-branch bf16 AllReduce(add) within the
    4-core batch group (branches 0/1 overlap later attention; branch 2 is
    split into two channel chunks pipelined against h1+conv2).
  - conv2 computed locally: full 512-channel contraction, only the core's own
    128 output channels. No further collective.
Host assembles the full (2, 512, 32, 32) output from the 8 per-core slabs.

v2 changes vs v1:
  - BN bias of each attention branch folded into the v/o projection bias
    host-side (y + b*denom = sum_k (u+b) p), so the post-softmax division is
    a single tensor_mul per head.
  - Softmax denominators: reciprocal_approx_fast (DVE) + gpsimd
    partition_broadcast (SBUF->SBUF) instead of full-precision DVE
    reciprocal + DMA roundtrip through DRAM.
  - y-matmul chains column-packed in pairs (tile_position (0,0)/(0,64)).
  - Per-branch AllReduce in bf16 instead of one fp32 AllReduce at the end.
  - Consolidated input DMAs.
"""

import os
import sys

import ml_dtypes
import numpy as np

for _p in ("/opt/trn_rl_repo",):
    if _p not in sys.path and os.path.isdir(_p):
        sys.path.insert(0, _p)

import concourse.bass as bass
import concourse.mybir as mybir
import concourse.tile as tile
from concourse import bacc
from concourse.bass_utils import run_bass_kernel_spmd

F32 = mybir.dt.float32
BF16 = mybir.dt.bfloat16
AF = mybir.ActivationFunctionType

# The ACT table-set picker is greedy-first-match: with both Exp and Ln in the
# kernel it alternates exp_and_others <-> natural_log (~2.7us per reload, ~25
# reloads).  Restrict matching to the one set that contains every function we
# use (exp, ln, relu, copy) so exactly one table load is emitted.  Keyed by
# name, dict length/order preserved so set ids stay valid.
_ACT_KEEP_SET = "natural_log_exp_and_others"
_orig_get_act_tables = bacc.get_activation_tables


def _patched_get_act_tables(arch):
    tabs = _orig_get_act_tables(arch)
    return {n: (fns if n == _ACT_KEEP_SET else set()) for n, fns in tabs.items()}


bacc.get_activation_tables = _patched_get_act_tables
B, C, H, W, HEADS = 2, 512, 32, 32, 16
D = C // HEADS            # 32
S = H * W                 # 1024
EPS = 1e-5
ISQD = 1.0 / np.sqrt(D)   # folded into the exp activation
NCORES = 8
GROUPS = [[0, 1, 2, 3], [4, 5, 6, 7]]
HP = W + 2                # padded row stride (34)


def build_nc():
    nc = bacc.Bacc(None, target_bir_lowering=False)

    # ---- per-core external inputs -------------------------------------
    x4_d = nc.declare_dram_parameter("x4", [4, 128, S], BF16, isOutput=False)
    oth_d = nc.declare_dram_parameter("oth", [2, 4, 128, S], BF16, isOutput=False)
    wqT_d = nc.declare_dram_parameter("wqT", [3, 4, 128, 128], BF16, isOutput=False)
    wkT_d = nc.declare_dram_parameter("wkT", [3, 4, 128, 128], BF16, isOutput=False)
    wvoT_d = nc.declare_dram_parameter("wvoT", [4, 128, 384], BF16, isOutput=False)
    dvec_d = nc.declare_dram_parameter("dvec", [128, 6], F32, isOutput=False)
    wobv_d = nc.declare_dram_parameter("wobv", [1, 384], F32, isOutput=False)
    c1wT_d = nc.declare_dram_parameter("c1wT", [3, 4, 128, 9, 128], BF16, isOutput=False)
    c2wT_d = nc.declare_dram_parameter("c2wT", [4, 128, 9, 128], BF16, isOutput=False)
    avec_d = nc.declare_dram_parameter("avec", [128, 10], F32, isOutput=False)
    out_d = nc.declare_dram_parameter("out", [128, S], F32, isOutput=True)

    with tile.TileContext(nc) as tc:
        import contextlib

        ctx = contextlib.ExitStack()
        with ctx:
            const = ctx.enter_context(tc.tile_pool(name="const", bufs=1))
            kq = ctx.enter_context(tc.tile_pool(name="kq", bufs=1))
            xtp = ctx.enter_context(tc.tile_pool(name="xtp", bufs=1))
            stg = ctx.enter_context(tc.tile_pool(name="stg", bufs=1))
            brp = ctx.enter_context(tc.tile_pool(name="brp", bufs=2))
            rcp = ctx.enter_context(tc.tile_pool(name="rcp", bufs=2))
            scps = ctx.enter_context(tc.tile_pool(name="scps", bufs=2, space="PSUM"))
            yps = ctx.enter_context(tc.tile_pool(name="yps", bufs=1, space="PSUM"))
            smps = ctx.enter_context(tc.tile_pool(name="smps", bufs=2, space="PSUM"))
            dram = ctx.enter_context(tc.tile_pool(name="dram", bufs=1, space="DRAM"))
            dramw = ctx.enter_context(tc.tile_pool(name="dramw", bufs=4, space="DRAM"))

            # ---- activations + branch-0 weights first (DMA priority) -----
            x_sb = const.tile([128, 4, S], BF16)
            nc.sync.dma_start(out=x_sb, in_=x4_d[:].rearrange("k p s -> p k s"))
            wq_sb = const.tile([128, 3, 4, 128], BF16)
            wk_sb = const.tile([128, 3, 4, 128], BF16)
            nc.sync.dma_start(out=wk_sb, in_=wkT_d[:].rearrange("i k p f -> p i k f"))
            nc.sync.dma_start(out=wq_sb, in_=wqT_d[:].rearrange("i k p f -> p i k f"))
            othp = ctx.enter_context(tc.tile_pool(name="othp", bufs=1))
            oth1 = othp.tile([128, 4, S], BF16, name="oth")
            nc.sync.dma_start(out=oth1, in_=oth_d[0].rearrange("k p s -> p k s"))
            wvo_sb = const.tile([128, 4, 384], BF16)
            nc.sync.dma_start(out=wvo_sb, in_=wvoT_d[:].rearrange("k p f -> p k f"))

            # Small consts: DMA to staging, then re-own on the consuming
            # engine (DVE / ACT) so consumers need no cross-engine const wait.
            dvec_st = const.tile([128, 6], F32)
            nc.gpsimd.dma_start(out=dvec_st, in_=dvec_d[:])
            wobv_st = const.tile([128, 384], F32)
            nc.gpsimd.dma_start(out=wobv_st, in_=wobv_d[:].partition_broadcast(128))
            avec_st = const.tile([128, 10], F32)
            nc.gpsimd.dma_start(out=avec_st, in_=avec_d[:])
            dvec = const.tile([128, 6], F32)
            nc.vector.tensor_copy(dvec, dvec_st)
            wobv_sb = const.tile([128, 384], F32)
            nc.vector.tensor_copy(wobv_sb, wobv_st)
            avec = const.tile([128, 10], F32)
            nc.scalar.activation(out=avec, in_=avec_st, func=AF.Copy)
            bqv_sb = dvec[:, 0:3]
            bkv_sb = dvec[:, 3:6]

            # xt (attention output) slabs + h1 slabs, zero-padded 34x34
            xt_sl = []
            for i in range(3):
                t = xtp.tile([128, HP, HP], BF16, name=f"xt{i}")
                nc.vector.memset(t, 0.0)
                xt_sl.append(t)
            h1_sl = []
            for k in range(4):
                t = xtp.tile([128, HP, HP], BF16, name=f"h1{k}")
                nc.vector.memset(t, 0.0)
                h1_sl.append(t)
            # per-branch conv1 partials (f32 staging for the AllReduce);
            # 2 rotating slots — branch i is shipped before i+1 is written
            brst = {}

            def get_brst(i):
                if i not in brst:
                    brst[i] = brp.tile([128, 4, S], F32, name="brst")
                return brst[i]

            # Semaphore warmers: absorb const-DMA + memset waits into each
            # engine's observed clock so later compute ops need <=1 wait.
            warm = const.tile([128, 1], F32)
            nc.vector.tensor_copy(warm, dvec[:, 0:1])
            warm2 = const.tile([128, 1], F32)
            nc.scalar.activation(out=warm2, in_=warm, func=AF.Copy)

            # k/q per branch (with biases added), uT tiles
            k_sb = kq.tile([128, 3, S], BF16)
            q_sb = kq.tile([128, 3, S], BF16)
            uT = [kq.tile([128, 3, 4, 33], BF16, name=f"uT{t}") for t in range(8)]

            # ---- projections (k0/q0 upfront; rest fill branch-0 slots) ---
            # x_prev reuses x_next's slot once q_proj(0) has consumed it
            qsrc = [oth1, x_sb, None]

            def load_oth2():
                oth2 = othp.tile([128, 4, S], BF16, name="oth")
                nc.sync.dma_start(out=oth2, in_=oth_d[1].rearrange("k p s -> p k s"))
                qsrc[2] = oth2

            def k_proj(i):
                k_ps = scps.tile([128, S], F32, name="kq_ps", tag="sc")
                for s in range(2):
                    for ks in range(4):
                        nc.tensor.matmul(
                            k_ps[:, 512 * s : 512 * (s + 1)],
                            lhsT=wk_sb[:, i, ks, :],
                            rhs=x_sb[:, ks, 512 * s : 512 * (s + 1)],
                            start=(ks == 0),
                            stop=(ks == 3),
                        )
                nc.vector.tensor_scalar_add(k_sb[:, i, :], k_ps, bkv_sb[:, i : i + 1])

            def q_proj(i):
                q_ps = scps.tile([128, S], F32, name="kq_ps2", tag="sc")
                for s in range(2):
                    for ks in range(4):
                        nc.tensor.matmul(
                            q_ps[:, 512 * s : 512 * (s + 1)],
                            lhsT=wq_sb[:, i, ks, :],
                            rhs=qsrc[i][:, ks, 512 * s : 512 * (s + 1)],
                            start=(ks == 0),
                            stop=(ks == 3),
                        )
                nc.vector.tensor_scalar_add(q_sb[:, i, :], q_ps, bqv_sb[:, i : i + 1])

            def u_proj(t):
                u_ps = smps.tile([128, 384], F32, name="u_ps", tag="sm")
                for ks in range(4):
                    nc.tensor.matmul(
                        u_ps,
                        lhsT=x_sb[:, ks, 128 * t : 128 * (t + 1)],
                        rhs=wvo_sb[:, ks, :],
                        start=(ks == 0),
                        stop=(ks == 3),
                    )
                nc.vector.memset(uT[t][:, :, :, 32:33], 1.0)
                # wobv has the attention-BN bias folded in host-side
                nc.vector.tensor_add(
                    uT[t][:, :, :, 0:32],
                    u_ps.rearrange("p (i h d) -> p i h d", i=3, h=4),
                    wobv_sb.rearrange("p (i h d) -> p i h d", i=3, h=4),
                )

            k_proj(0)
            q_proj(0)
            load_oth2()

            # ---- conv weights (emitted after proj psum freed) ------------
            convw = ctx.enter_context(tc.tile_pool(name="convw", bufs=1))
            pt = ctx.enter_context(tc.tile_pool(name="pt", bufs=16))
            c1w_sb = [
                [convw.tile([128, 9, 128], BF16, name=f"c1w{i}_{m}") for m in range(4)]
                for i in range(3)
            ]
            for i in range(3):
                for m in range(4):
                    nc.sync.dma_start(out=c1w_sb[i][m], in_=c1wT_d[i, m])
            c2w_sb = [convw.tile([128, 9, 128], BF16, name=f"c2w{k}") for k in range(4)]
            for k in range(4):
                nc.sync.dma_start(out=c2w_sb[k], in_=c2wT_d[k])

            def conv1_block(i, m, n):
                """Partial conv1 for xt slab i, out m-tile, spatial half n,
                written (bf16) into brst[i]."""
                ps = smps.tile([128, 512], F32, name="cv", tag="sm")
                for dy in range(3):
                    for dx in range(3):
                        nc.tensor.matmul(
                            ps,
                            lhsT=c1w_sb[i][m][:, dy * 3 + dx, :],
                            rhs=xt_sl[i][:, 16 * n + dy : 16 * n + dy + 16, dx : dx + 32],
                            start=(dy == 0 and dx == 0),
                            stop=(dy == 2 and dx == 2),
                        )
                nc.vector.tensor_copy(get_brst(i)[:, m, 512 * n : 512 * (n + 1)], ps)
                if n == 1:
                    if i < 2:
                        dst = partial[i][128 * m : 128 * (m + 1), :]
                    else:
                        dst = partial2[m // 2][128 * (m % 2) : 128 * (m % 2) + 128, :]
                    nc.sync.dma_start(out=dst, in_=get_brst(i)[:, m, :])

            def attention(i, pr, filler):
                """Heads (2pr, 2pr+1) of branch i.  `filler` is a list of
                thunks (conv1 blocks / projections) sprinkled between the
                per-t score groups to keep PE dense while ACT grinds exps."""
                heads = (2 * pr, 2 * pr + 1)
                pts = {}
                fi = 0
                for t in range(8):
                    for h in heads:
                        sc = scps.tile([128, S], F32, name="sc", tag="sc")
                        p0 = 32 * h
                        for s in range(2):
                            nc.tensor.matmul(
                                sc[:, 512 * s : 512 * (s + 1)],
                                lhsT=k_sb[p0 : p0 + 32, i, 128 * t : 128 * (t + 1)],
                                rhs=q_sb[p0 : p0 + 32, i, 512 * s : 512 * (s + 1)],
                                start=True,
                                stop=True,
                                tile_position=(p0, 0),
                            )
                        ptt = pt.tile([128, S], BF16, name="ptt")
                        nc.scalar.activation(
                            out=ptt, in_=sc, func=AF.Exp, scale=float(ISQD)
                        )
                        pts[(h, t)] = ptt
                    while fi < len(filler) * (t + 1) // 8:
                        filler[fi]()
                        fi += 1
                # y chains: per head, the two query-half chains target the
                # two zero regions of one [33, S] psum tile sequentially
                for h in heads:
                    y2 = yps.tile([33, S], F32, name="y2", tag="y")
                    for s in range(2):
                        for t in range(8):
                            nc.tensor.matmul(
                                y2[:, 512 * s : 512 * (s + 1)],
                                lhsT=uT[t][:, i, h, :],
                                rhs=pts[(h, t)][:, 512 * s : 512 * (s + 1)],
                                start=(t == 0),
                                stop=(t == 7),
                            )
                    p0 = 32 * h
                    # 1/denom as exp(-ln(denom)) on ACT: ln and exp share the
                    # natural_log_exp_and_others table set (no reload), and
                    # both DVE reciprocal (6.5ns/elem) and the custom-DVE
                    # approx op (garbage on this HW) are avoided.
                    rc = rcp.tile([1, S], F32, name="rc")
                    nc.scalar.activation(out=rc, in_=y2[32:33, :], func=AF.Ln)
                    nc.scalar.activation(out=rc, in_=rc, func=AF.Exp, scale=-1.0)
                    rcb = rcp.tile([32, S], F32, name="rcb")
                    nc.gpsimd.partition_broadcast(rcb, rc[:])
                    nc.vector.tensor_mul(
                        xt_sl[i][p0 : p0 + 32, 1:33, 1:33],
                        y2[0:32, :].rearrange("p (a b) -> p a b", b=32),
                        rcb.rearrange("p (a b) -> p a b", b=32),
                    )
                while fi < len(filler):
                    filler[fi]()
                    fi += 1

            # ---- collectives ---------------------------------------------
            # branch 0/1: one [512, S] bf16 AllReduce each, overlapped with
            # the next branch's attention.  branch 2: two [256, S] chunks
            # (m01 / m23) pipelined against h1+conv2.
            partial = [dram.tile([512, S], F32, name=f"part{i}") for i in range(2)]
            art = [dram.tile([512, S], F32, name=f"art{i}") for i in range(2)]
            partial2 = [dram.tile([256, S], F32, name=f"part2{a}") for a in range(2)]
            art2 = [dram.tile([256, S], F32, name=f"art2{a}") for a in range(2)]

            def ar_branch(i):
                nc.gpsimd.collective_compute(
                    "AllReduce",
                    mybir.AluOpType.add,
                    replica_groups=GROUPS,
                    ins=[partial[i][:]],
                    outs=[art[i][:]],
                )

            def ar2_chunk(a):
                nc.gpsimd.collective_compute(
                    "AllReduce",
                    mybir.AluOpType.add,
                    replica_groups=GROUPS,
                    ins=[partial2[a][:]],
                    outs=[art2[a][:]],
                )

            def conv1_and_ship(i):
                return [
                    (lambda m=m, n=n: conv1_block(i, m, n))
                    for m in range(4)
                    for n in range(2)
                ]

            # ---- phase A: attention with projections/conv1 interleaved ---
            attention(0, 0, [lambda t=t: u_proj(t) for t in range(8)])
            attention(0, 1, [lambda: k_proj(1), lambda: q_proj(1),
                             lambda: k_proj(2), lambda: q_proj(2)])
            f0 = conv1_and_ship(0)
            attention(1, 0, f0[:4])
            attention(1, 1, f0[4:] + [lambda: ar_branch(0)])
            f1 = conv1_and_ship(1)
            attention(2, 0, f1[:4])
            attention(2, 1, f1[4:] + [lambda: ar_branch(1)])
            # branch 2 conv1 + chunked AR at the end
            for m in range(4):
                for n in range(2):
                    conv1_block(2, m, n)
                if m == 1:
                    ar2_chunk(0)
            ar2_chunk(1)

            # ---- phase C: combine, BN1+relu, conv2, BN2+relu, out --------
            arr01 = stg.tile([128, 4, S], BF16, name="arr01", bufs=1)
            arrt = [stg.tile([128, 2, S], F32, name=f"arrt{j}", bufs=1) for j in range(2)]
            # art0 + art1 (during late attention / AR2 flight)
            nc.gpsimd.dma_start(
                out=arr01, in_=art[0][:].rearrange("(m p) s -> p m s", p=128)
            )
            nc.gpsimd.dma_start(
                out=arrt[0],
                in_=art[1][:].rearrange("(m p) s -> p m s", p=128)[:, 0:2, :],
            )
            nc.gpsimd.dma_start(
                out=arrt[1],
                in_=art[1][:].rearrange("(m p) s -> p m s", p=128)[:, 2:4, :],
            )
            nc.vector.tensor_add(
                arr01[:, 0:2, :], arr01[:, 0:2, :], arrt[0][:, 0:2, :]
            )
            nc.vector.tensor_add(
                arr01[:, 2:4, :], arr01[:, 2:4, :], arrt[1][:, 0:2, :]
            )

            oout = stg.tile([128, S], F32, name="oout", bufs=1)
            ps2 = [smps.tile([128, 512], F32, name=f"cv2_{n}", tag="sm") for n in range(2)]

            def h1_chunk(a):
                """arr01[m01/m23] + art2 chunk -> BN1+relu -> h1 slabs."""
                nc.gpsimd.dma_start(
                    out=arrt[0],
                    in_=art2[a][:].rearrange("(m p) s -> p m s", p=128),
                )
                src = arrt[0]
                nc.vector.tensor_add(
                    src[:, 0:2, :], src[:, 0:2, :], arr01[:, 2 * a : 2 * a + 2, :]
                )
                for j, k in enumerate((2 * a, 2 * a + 1)):
                    nc.scalar.activation(
                        out=h1_sl[k][:, 1:33, 1:33],
                        in_=src[:, j, :].rearrange("p (a b) -> p a b", b=32),
                        func=AF.Relu,
                        bias=avec[:, 4 + k : 5 + k],
                        scale=avec[:, k : k + 1],
                    )

            def conv2_half(a):
                # accumulate k-slabs 2a, 2a+1 into both spatial halves
                for n in range(2):
                    for k in (2 * a, 2 * a + 1):
                        for dy in range(3):
                            for dx in range(3):
                                nc.tensor.matmul(
                                    ps2[n],
                                    lhsT=c2w_sb[k][:, dy * 3 + dx, :],
                                    rhs=h1_sl[k][
                                        :, 16 * n + dy : 16 * n + dy + 16, dx : dx + 32
                                    ],
                                    start=(k == 0 and dy == 0 and dx == 0),
                                    stop=(k == 3 and dy == 2 and dx == 2),
                                )

            h1_chunk(0)
            conv2_half(0)   # overlaps AR2 chunk 1
            h1_chunk(1)
            conv2_half(1)
            for n in range(2):
                nc.scalar.activation(
                    out=oout[:, 512 * n : 512 * (n + 1)],
                    in_=ps2[n],
                    func=AF.Relu,
                    bias=avec[:, 9:10],
                    scale=avec[:, 8:9],
                )
                nc.sync.dma_start(
                    out=out_d[:, 512 * n : 512 * (n + 1)],
                    in_=oout[:, 512 * n : 512 * (n + 1)],
                )

    nc.finalize()
    return nc


def _f(x):
    return np.ascontiguousarray(x, dtype=np.float32)


def _bf(x):
    return np.ascontiguousarray(np.asarray(x, dtype=np.float32).astype(ml_dtypes.bfloat16))


def prepare_core_inputs(inp):
    """Build the 8 per-core input dicts from the full-problem inputs."""
    inp = {k: np.asarray(v, dtype=np.float64) for k, v in inp.items()}
    x = inp["x"].reshape(B, C, S)
    xp = inp["x_prev"].reshape(B, C, S)
    xn = inp["x_next"].reshape(B, C, S)

    bn1s_full = inp["bn1g"] / np.sqrt(inp["bn1v"] + EPS)
    bn1b_full = inp["bn1b"] - inp["bn1m"] * bn1s_full
    bn2s_full = inp["bn2g"] / np.sqrt(inp["bn2v"] + EPS)
    bn2b_full = inp["bn2b"] - inp["bn2m"] * bn2s_full

    per_g = []
    for g in range(4):
        sl = slice(128 * g, 128 * (g + 1))
        wqT = np.stack(
            [
                np.stack([inp["Wq"][i][sl, 128 * k : 128 * (k + 1)].T for k in range(4)])
                for i in range(3)
            ]
        )
        wkT = np.stack(
            [
                np.stack([inp["Wk"][i][sl, 128 * k : 128 * (k + 1)].T for k in range(4)])
                for i in range(3)
            ]
        )
        bqv = np.stack([inp["bq"][i][sl] for i in range(3)], axis=1)
        bkv = np.stack([inp["bk"][i][sl] for i in range(3)], axis=1)

        att_s = np.stack(
            [inp["bng"][i][sl] / np.sqrt(inp["bnv"][i][sl] + EPS) for i in range(3)]
        )  # (3,128)
        xtb = np.stack(
            [
                inp["bnb"][i][sl] + (inp["bo"][i][sl] - inp["bnm"][i][sl]) * att_s[i]
                for i in range(3)
            ]
        )  # (3,128)

        wvo_rows = []
        wobv_row = []
        for i in range(3):
            for hl in range(4):
                hg = 4 * g + hl
                wv_h = inp["Wv"][i][32 * hg : 32 * (hg + 1), :]  # (32, 512)
                bv_h = inp["bv"][i][32 * hg : 32 * (hg + 1)]
                wo_h = inp["Wo"][i, hg]  # (32, 32)
                sc = att_s[i][32 * hl : 32 * (hl + 1)]  # (32,)
                wvo_rows.append(sc[:, None] * (wo_h @ wv_h))
                # fold the (BN-scaled) output bias + BN bias into the u bias:
                # y/denom + xtb == sum_k (u + xtb) p_k / denom
                wobv_row.append(sc * (wo_h @ bv_h) + xtb[i][32 * hl : 32 * (hl + 1)])
        wvo_all = np.concatenate(wvo_rows, axis=0)  # (384, 512)
        wobv = np.concatenate(wobv_row)[None, :]  # (1, 384)
        wvoT = np.stack([wvo_all[:, 128 * k : 128 * (k + 1)].T for k in range(4)])

        c1wT = np.stack(
            [
                np.stack(
                    [
                        inp["c1w"][
                            128 * m : 128 * (m + 1),
                            512 * i + 128 * g : 512 * i + 128 * (g + 1),
                        ]
                        .transpose(1, 2, 3, 0)
                        .reshape(128, 9, 128)
                        for m in range(4)
                    ]
                )
                for i in range(3)
            ]
        )
        c2wT = np.stack(
            [
                inp["c2w"][sl, 128 * k : 128 * (k + 1)]
                .transpose(1, 2, 3, 0)
                .reshape(128, 9, 128)
                for k in range(4)
            ]
        )
        avec = np.concatenate(
            [
                bn1s_full.reshape(4, 128).T,
                bn1b_full.reshape(4, 128).T,
                bn2s_full[sl][:, None],
                bn2b_full[sl][:, None],
            ],
            axis=1,
        )  # (128, 10)

        per_g.append(
            dict(
                wqT=_bf(wqT), wkT=_bf(wkT), wvoT=_bf(wvoT),
                wobv=_f(wobv), c1wT=_bf(c1wT), c2wT=_bf(c2wT),
                dvec=_f(np.concatenate([bqv, bkv], axis=1)),
                avec=_f(avec),
            )
        )

    in_maps = []
    for c in range(NCORES):
        b, g = c // 4, c % 4
        d = dict(per_g[g])
        d["x4"] = _bf(x[b].reshape(4, 128, S))
        d["oth"] = _bf(np.stack([xn[b].reshape(4, 128, S), xp[b].reshape(4, 128, S)]))
        in_maps.append(d)
    return in_maps


_NC_CACHE = {}


def get_nc():
    if "nc" not in _NC_CACHE:
        _NC_CACHE["nc"] = build_nc()
    return _NC_CACHE["nc"]


def assemble(results):
    out = np.zeros((B, C, H, W), dtype=np.float32)
    for c in range(NCORES):
        b, g = c // 4, c % 4
        out[b, 128 * g : 128 * (g + 1)] = results[c]["out"].reshape(128, H, W)
    return out


def kernel(**inputs):
    nc = get_nc()
    in_maps = prepare_core_inputs(inputs)
    res = run_bass_kernel_spmd(nc, in_maps, list(range(NCORES)))
    return assemble(res.results)


# revision 26
# speedup vs baseline: 1.1932x; 1.0789x over previous
"""CSAEncoder Trainium2 kernel: 3-branch cross-attention + concat DoubleConv.

Sharding (8 cores): 2 batch groups x 4 tensor ranks.
Core c: batch b = c // 4, rank g = c % 4.
  - Attention: core computes heads [4g, 4g+4) of all 3 branches for batch b
    (a contiguous 128-channel slab of each branch's output).
  - conv1 computed as partial sums over the core's local 384 input channels
    for ALL 512 output channels; per-branch bf16 AllReduce(add) within the
    4-core batch group (branches 0/1 overlap later attention; branch 2 is
    split into two channel chunks pipelined against h1+conv2).
  - conv2 computed locally: full 512-channel contraction, only the core's own
    128 output channels. No further collective.
Host assembles the full (2, 512, 32, 32) output from the 8 per-core slabs.

v2 changes vs v1:
  - BN bias of each attention branch folded into the v/o projection bias
    host-side (y + b*denom = sum_k (u+b) p), so the post-softmax division is
    a single tensor_mul per head.
  - Softmax denominators: reciprocal_approx_fast (DVE) + gpsimd
    partition_broadcast (SBUF->SBUF) instead of full-precision DVE
    reciprocal + DMA roundtrip through DRAM.
  - y-matmul chains column-packed in pairs (tile_position (0,0)/(0,64)).
  - Per-branch AllReduce in bf16 instead of one fp32 AllReduce at the end.
  - Consolidated input DMAs.
"""

import os
import sys

import ml_dtypes
import numpy as np

for _p in ("/opt/trn_rl_repo",):
    if _p not in sys.path and os.path.isdir(_p):
        sys.path.insert(0, _p)

import concourse.bass as bass
import concourse.mybir as mybir
import concourse.tile as tile
from concourse import bacc
from concourse.bass_utils import run_bass_kernel_spmd

F32 = mybir.dt.float32
BF16 = mybir.dt.bfloat16
AF = mybir.ActivationFunctionType

# The ACT table-set picker is greedy-first-match: with both Exp and Ln in the
# kernel it alternates exp_and_others <-> natural_log (~2.7us per reload, ~25
# reloads).  Restrict matching to the one set that contains every function we
# use (exp, ln, relu, copy) so exactly one table load is emitted.  Keyed by
# name, dict length/order preserved so set ids stay valid.
_ACT_KEEP_SET = "natural_log_exp_and_others"
_orig_get_act_tables = bacc.get_activation_tables


def _patched_get_act_tables(arch):
    tabs = _orig_get_act_tables(arch)
    return {n: (fns if n == _ACT_KEEP_SET else set()) for n, fns in tabs.items()}


bacc.get_activation_tables = _patched_get_act_tables
B, C, H, W, HEADS = 2, 512, 32, 32, 16
D = C // HEADS            # 32
S = H * W                 # 1024
EPS = 1e-5
ISQD = 1.0 / np.sqrt(D)   # folded into the exp activation
NCORES = 8
GROUPS = [[0, 1, 2, 3], [4, 5, 6, 7]]
HP = W + 2                # padded row stride (34)


def build_nc():
    nc = bacc.Bacc(None, target_bir_lowering=False)

    # ---- per-core external inputs -------------------------------------
    x4_d = nc.declare_dram_parameter("x4", [4, 128, S], BF16, isOutput=False)
    oth_d = nc.declare_dram_parameter("oth", [2, 4, 128, S], BF16, isOutput=False)
    wqT_d = nc.declare_dram_parameter("wqT", [3, 4, 128, 128], BF16, isOutput=False)
    wkT_d = nc.declare_dram_parameter("wkT", [3, 4, 128, 128], BF16, isOutput=False)
    wvoT_d = nc.declare_dram_parameter("wvoT", [4, 128, 384], BF16, isOutput=False)
    dvec_d = nc.declare_dram_parameter("dvec", [128, 6], F32, isOutput=False)
    wobv_d = nc.declare_dram_parameter("wobv", [1, 384], F32, isOutput=False)
    c1wT_d = nc.declare_dram_parameter("c1wT", [3, 4, 128, 9, 128], BF16, isOutput=False)
    c2wT_d = nc.declare_dram_parameter("c2wT", [4, 128, 9, 128], BF16, isOutput=False)
    avec_d = nc.declare_dram_parameter("avec", [128, 10], F32, isOutput=False)
    out_d = nc.declare_dram_parameter("out", [128, S], F32, isOutput=True)

    with tile.TileContext(nc) as tc:
        import contextlib

        ctx = contextlib.ExitStack()
        with ctx:
            const = ctx.enter_context(tc.tile_pool(name="const", bufs=1))
            kq = ctx.enter_context(tc.tile_pool(name="kq", bufs=1))
            xtp = ctx.enter_context(tc.tile_pool(name="xtp", bufs=1))
            stg = ctx.enter_context(tc.tile_pool(name="stg", bufs=1))
            brp = ctx.enter_context(tc.tile_pool(name="brp", bufs=2))
            rcp = ctx.enter_context(tc.tile_pool(name="rcp", bufs=2))
            scps = ctx.enter_context(tc.tile_pool(name="scps", bufs=2, space="PSUM"))
            yps = ctx.enter_context(tc.tile_pool(name="yps", bufs=1, space="PSUM"))
            smps = ctx.enter_context(tc.tile_pool(name="smps", bufs=2, space="PSUM"))
            dram = ctx.enter_context(tc.tile_pool(name="dram", bufs=1, space="DRAM"))
            dramw = ctx.enter_context(tc.tile_pool(name="dramw", bufs=4, space="DRAM"))

            # ---- activations + branch-0 weights first (DMA priority) -----
            x_sb = const.tile([128, 4, S], BF16)
            nc.sync.dma_start(out=x_sb, in_=x4_d[:].rearrange("k p s -> p k s"))
            wq_sb = const.tile([128, 3, 4, 128], BF16)
            wk_sb = const.tile([128, 3, 4, 128], BF16)
            nc.sync.dma_start(out=wk_sb, in_=wkT_d[:].rearrange("i k p f -> p i k f"))
            nc.sync.dma_start(out=wq_sb, in_=wqT_d[:].rearrange("i k p f -> p i k f"))
            othp = ctx.enter_context(tc.tile_pool(name="othp", bufs=1))
            oth1 = othp.tile([128, 4, S], BF16, name="oth")
            nc.sync.dma_start(out=oth1, in_=oth_d[0].rearrange("k p s -> p k s"))
            wvo_sb = const.tile([128, 4, 384], BF16)
            nc.sync.dma_start(out=wvo_sb, in_=wvoT_d[:].rearrange("k p f -> p k f"))

            # Small consts: DMA to staging, then re-own on the consuming
            # engine (DVE / ACT) so consumers need no cross-engine const wait.
            dvec_st = const.tile([128, 6], F32)
            nc.gpsimd.dma_start(out=dvec_st, in_=dvec_d[:])
            wobv_st = const.tile([128, 384], F32)
            nc.gpsimd.dma_start(out=wobv_st, in_=wobv_d[:].partition_broadcast(128))
            avec_st = const.tile([128, 10], F32)
            nc.gpsimd.dma_start(out=avec_st, in_=avec_d[:])
            dvec = const.tile([128, 6], F32)
            nc.vector.tensor_copy(dvec, dvec_st)
            wobv_sb = const.tile([128, 384], F32)
            nc.vector.tensor_copy(wobv_sb, wobv_st)
            avec = const.tile([128, 10], F32)
            nc.scalar.activation(out=avec, in_=avec_st, func=AF.Copy)
            bqv_sb = dvec[:, 0:3]
            bkv_sb = dvec[:, 3:6]

            # xt (attention output) slabs + h1 slabs, zero-padded 34x34
            xt_sl = []
            for i in range(3):
                t = xtp.tile([128, HP, HP], BF16, name=f"xt{i}")
                nc.vector.memset(t, 0.0)
                xt_sl.append(t)
            h1_sl = []
            for k in range(4):
                t = xtp.tile([128, HP, HP], BF16, name=f"h1{k}")
                nc.vector.memset(t, 0.0)
                h1_sl.append(t)
            # per-branch conv1 partials (f32 staging for the AllReduce);
            # 2 rotating slots — branch i is shipped before i+1 is written
            brst = {}

            def get_brst(i):
                if i not in brst:
                    brst[i] = brp.tile([128, 4, S], BF16, name="brst")
                return brst[i]

            # Semaphore warmers: absorb const-DMA + memset waits into each
            # engine's observed clock so later compute ops need <=1 wait.
            warm = const.tile([128, 1], F32)
            nc.vector.tensor_copy(warm, dvec[:, 0:1])
            warm2 = const.tile([128, 1], F32)
            nc.scalar.activation(out=warm2, in_=warm, func=AF.Copy)

            # k/q per branch (with biases added), uT tiles
            k_sb = kq.tile([128, 3, S], BF16)
            q_sb = kq.tile([128, 3, S], BF16)
            uT = [kq.tile([128, 3, 4, 33], BF16, name=f"uT{t}") for t in range(8)]

            # ---- projections (k0/q0 upfront; rest fill branch-0 slots) ---
            # x_prev reuses x_next's slot once q_proj(0) has consumed it
            qsrc = [oth1, x_sb, None]

            def load_oth2():
                oth2 = othp.tile([128, 4, S], BF16, name="oth")
                nc.sync.dma_start(out=oth2, in_=oth_d[1].rearrange("k p s -> p k s"))
                qsrc[2] = oth2

            def k_proj(i):
                k_ps = scps.tile([128, S], F32, name="kq_ps", tag="sc")
                for s in range(2):
                    for ks in range(4):
                        nc.tensor.matmul(
                            k_ps[:, 512 * s : 512 * (s + 1)],
                            lhsT=wk_sb[:, i, ks, :],
                            rhs=x_sb[:, ks, 512 * s : 512 * (s + 1)],
                            start=(ks == 0),
                            stop=(ks == 3),
                        )
                nc.vector.tensor_scalar_add(k_sb[:, i, :], k_ps, bkv_sb[:, i : i + 1])

            def q_proj(i):
                q_ps = scps.tile([128, S], F32, name="kq_ps2", tag="sc")
                for s in range(2):
                    for ks in range(4):
                        nc.tensor.matmul(
                            q_ps[:, 512 * s : 512 * (s + 1)],
                            lhsT=wq_sb[:, i, ks, :],
                            rhs=qsrc[i][:, ks, 512 * s : 512 * (s + 1)],
                            start=(ks == 0),
                            stop=(ks == 3),
                        )
                nc.vector.tensor_scalar_add(q_sb[:, i, :], q_ps, bqv_sb[:, i : i + 1])

            def u_proj(t):
                u_ps = smps.tile([128, 384], F32, name="u_ps", tag="sm")
                for ks in range(4):
                    nc.tensor.matmul(
                        u_ps,
                        lhsT=x_sb[:, ks, 128 * t : 128 * (t + 1)],
                        rhs=wvo_sb[:, ks, :],
                        start=(ks == 0),
                        stop=(ks == 3),
                    )
                nc.vector.memset(uT[t][:, :, :, 32:33], 1.0)
                # wobv has the attention-BN bias folded in host-side
                nc.vector.tensor_add(
                    uT[t][:, :, :, 0:32],
                    u_ps.rearrange("p (i h d) -> p i h d", i=3, h=4),
                    wobv_sb.rearrange("p (i h d) -> p i h d", i=3, h=4),
                )

            k_proj(0)
            q_proj(0)
            load_oth2()

            # ---- conv weights (emitted after proj psum freed) ------------
            convw = ctx.enter_context(tc.tile_pool(name="convw", bufs=1))
            pt = ctx.enter_context(tc.tile_pool(name="pt", bufs=16))
            c1w_sb = [
                [convw.tile([128, 9, 128], BF16, name=f"c1w{i}_{m}") for m in range(4)]
                for i in range(3)
            ]
            for i in range(3):
                for m in range(4):
                    nc.sync.dma_start(out=c1w_sb[i][m], in_=c1wT_d[i, m])
            c2w_sb = [convw.tile([128, 9, 128], BF16, name=f"c2w{k}") for k in range(4)]
            for k in range(4):
                nc.sync.dma_start(out=c2w_sb[k], in_=c2wT_d[k])

            def conv1_block(i, m, n):
                """Partial conv1 for xt slab i, out m-tile, spatial half n,
                written (bf16) into brst[i]."""
                ps = smps.tile([128, 512], F32, name="cv", tag="sm")
                for dy in range(3):
                    for dx in range(3):
                        nc.tensor.matmul(
                            ps,
                            lhsT=c1w_sb[i][m][:, dy * 3 + dx, :],
                            rhs=xt_sl[i][:, 16 * n + dy : 16 * n + dy + 16, dx : dx + 32],
                            start=(dy == 0 and dx == 0),
                            stop=(dy == 2 and dx == 2),
                        )
                nc.vector.tensor_copy(get_brst(i)[:, m, 512 * n : 512 * (n + 1)], ps)
                if n == 1:
                    if i < 2:
                        dst = partial[i][128 * m : 128 * (m + 1), :]
                    else:
                        dst = partial2[m // 2][128 * (m % 2) : 128 * (m % 2) + 128, :]
                    nc.sync.dma_start(out=dst, in_=get_brst(i)[:, m, :])

            def attention(i, pr, filler):
                """Heads (2pr, 2pr+1) of branch i.  `filler` is a list of
                thunks (conv1 blocks / projections) sprinkled between the
                per-t score groups to keep PE dense while ACT grinds exps."""
                heads = (2 * pr, 2 * pr + 1)
                pts = {}
                fi = 0
                for t in range(8):
                    for h in heads:
                        sc = scps.tile([128, S], F32, name="sc", tag="sc")
                        p0 = 32 * h
                        for s in range(2):
                            nc.tensor.matmul(
                                sc[:, 512 * s : 512 * (s + 1)],
                                lhsT=k_sb[p0 : p0 + 32, i, 128 * t : 128 * (t + 1)],
                                rhs=q_sb[p0 : p0 + 32, i, 512 * s : 512 * (s + 1)],
                                start=True,
                                stop=True,
                                tile_position=(p0, 0),
                            )
                        ptt = pt.tile([128, S], BF16, name="ptt")
                        nc.scalar.activation(
                            out=ptt, in_=sc, func=AF.Exp, scale=float(ISQD)
                        )
                        pts[(h, t)] = ptt
                    while fi < len(filler) * (t + 1) // 8:
                        filler[fi]()
                        fi += 1
                # y chains: per head, the two query-half chains target the
                # two zero regions of one [33, S] psum tile sequentially
                for h in heads:
                    y2 = yps.tile([33, S], F32, name="y2", tag="y")
                    for s in range(2):
                        for t in range(8):
                            nc.tensor.matmul(
                                y2[:, 512 * s : 512 * (s + 1)],
                                lhsT=uT[t][:, i, h, :],
                                rhs=pts[(h, t)][:, 512 * s : 512 * (s + 1)],
                                start=(t == 0),
                                stop=(t == 7),
                            )
                    p0 = 32 * h
                    # 1/denom as exp(-ln(denom)) on ACT: ln and exp share the
                    # natural_log_exp_and_others table set (no reload), and
                    # both DVE reciprocal (6.5ns/elem) and the custom-DVE
                    # approx op (garbage on this HW) are avoided.
                    rc = rcp.tile([1, S], F32, name="rc")
                    nc.scalar.activation(out=rc, in_=y2[32:33, :], func=AF.Ln)
                    nc.scalar.activation(out=rc, in_=rc, func=AF.Exp, scale=-1.0)
                    rcb = rcp.tile([32, S], F32, name="rcb")
                    nc.gpsimd.partition_broadcast(rcb, rc[:])
                    nc.vector.tensor_mul(
                        xt_sl[i][p0 : p0 + 32, 1:33, 1:33],
                        y2[0:32, :].rearrange("p (a b) -> p a b", b=32),
                        rcb.rearrange("p (a b) -> p a b", b=32),
                    )
                while fi < len(filler):
                    filler[fi]()
                    fi += 1

            # ---- collectives ---------------------------------------------
            # branch 0/1: one [512, S] bf16 AllReduce each, overlapped with
            # the next branch's attention.  branch 2: two [256, S] chunks
            # (m01 / m23) pipelined against h1+conv2.
            partial = [dram.tile([512, S], BF16, name=f"part{i}") for i in range(2)]
            art = [dram.tile([512, S], BF16, name=f"art{i}") for i in range(2)]
            partial2 = [dram.tile([256, S], BF16, name=f"part2{a}") for a in range(2)]
            art2 = [dram.tile([256, S], BF16, name=f"art2{a}") for a in range(2)]

            def ar_branch(i):
                nc.gpsimd.collective_compute(
                    "AllReduce",
                    mybir.AluOpType.add,
                    replica_groups=GROUPS,
                    ins=[partial[i][:]],
                    outs=[art[i][:]],
                )

            def ar2_chunk(a):
                nc.gpsimd.collective_compute(
                    "AllReduce",
                    mybir.AluOpType.add,
                    replica_groups=GROUPS,
                    ins=[partial2[a][:]],
                    outs=[art2[a][:]],
                )

            def conv1_and_ship(i):
                return [
                    (lambda m=m, n=n: conv1_block(i, m, n))
                    for m in range(4)
                    for n in range(2)
                ]

            # ---- phase A: attention with projections/conv1 interleaved ---
            attention(0, 0, [lambda t=t: u_proj(t) for t in range(8)])
            attention(0, 1, [lambda: k_proj(1), lambda: q_proj(1),
                             lambda: k_proj(2), lambda: q_proj(2)])
            f0 = conv1_and_ship(0)
            attention(1, 0, f0[:4])
            attention(1, 1, f0[4:] + [lambda: ar_branch(0)])
            f1 = conv1_and_ship(1)
            attention(2, 0, f1[:4])
            attention(2, 1, f1[4:] + [lambda: ar_branch(1)])
            # branch 2 conv1 + chunked AR at the end
            for m in range(4):
                for n in range(2):
                    conv1_block(2, m, n)
                if m == 1:
                    ar2_chunk(0)
            ar2_chunk(1)

            # ---- phase C: combine, BN1+relu, conv2, BN2+relu, out --------
            arr01 = stg.tile([128, 4, S], BF16, name="arr01", bufs=1)
            arrt = [stg.tile([128, 2, S], BF16, name=f"arrt{j}", bufs=1) for j in range(2)]
            # art0 + art1 (during late attention / AR2 flight)
            nc.gpsimd.dma_start(
                out=arr01, in_=art[0][:].rearrange("(m p) s -> p m s", p=128)
            )
            nc.gpsimd.dma_start(
                out=arrt[0],
                in_=art[1][:].rearrange("(m p) s -> p m s", p=128)[:, 0:2, :],
            )
            nc.gpsimd.dma_start(
                out=arrt[1],
                in_=art[1][:].rearrange("(m p) s -> p m s", p=128)[:, 2:4, :],
            )
            nc.vector.tensor_add(
                arr01[:, 0:2, :], arr01[:, 0:2, :], arrt[0][:, 0:2, :]
            )
            nc.vector.tensor_add(
                arr01[:, 2:4, :], arr01[:, 2:4, :], arrt[1][:, 0:2, :]
            )

            oout = stg.tile([128, S], F32, name="oout", bufs=1)
            ps2 = [smps.tile([128, 512], F32, name=f"cv2_{n}", tag="sm") for n in range(2)]

            def h1_chunk(a):
                """arr01[m01/m23] + art2 chunk -> BN1+relu -> h1 slabs."""
                nc.gpsimd.dma_start(
                    out=arrt[0],
                    in_=art2[a][:].rearrange("(m p) s -> p m s", p=128),
                )
                src = arrt[0]
                nc.vector.tensor_add(
                    src[:, 0:2, :], src[:, 0:2, :], arr01[:, 2 * a : 2 * a + 2, :]
                )
                for j, k in enumerate((2 * a, 2 * a + 1)):
                    nc.scalar.activation(
                        out=h1_sl[k][:, 1:33, 1:33],
                        in_=src[:, j, :].rearrange("p (a b) -> p a b", b=32),
                        func=AF.Relu,
                        bias=avec[:, 4 + k : 5 + k],
                        scale=avec[:, k : k + 1],
                    )

            def conv2_half(a):
                # accumulate k-slabs 2a, 2a+1 into both spatial halves
                for n in range(2):
                    for k in (2 * a, 2 * a + 1):
                        for dy in range(3):
                            for dx in range(3):
                                nc.tensor.matmul(
                                    ps2[n],
                                    lhsT=c2w_sb[k][:, dy * 3 + dx, :],
                                    rhs=h1_sl[k][
                                        :, 16 * n + dy : 16 * n + dy + 16, dx : dx + 32
                                    ],
                                    start=(k == 0 and dy == 0 and dx == 0),
                                    stop=(k == 3 and dy == 2 and dx == 2),
                                )

            h1_chunk(0)
            conv2_half(0)   # overlaps AR2 chunk 1
            h1_chunk(1)
            conv2_half(1)
            for n in range(2):
                nc.scalar.activation(
                    out=oout[:, 512 * n : 512 * (n + 1)],
                    in_=ps2[n],
                    func=AF.Relu,
                    bias=avec[:, 9:10],
                    scale=avec[:, 8:9],
                )
                nc.sync.dma_start(
                    out=out_d[:, 512 * n : 512 * (n + 1)],
                    in_=oout[:, 512 * n : 512 * (n + 1)],
                )

    nc.finalize()
    return nc


def _f(x):
    return np.ascontiguousarray(x, dtype=np.float32)


def _bf(x):
    return np.ascontiguousarray(np.asarray(x, dtype=np.float32).astype(ml_dtypes.bfloat16))


def prepare_core_inputs(inp):
    """Build the 8 per-core input dicts from the full-problem inputs."""
    inp = {k: np.asarray(v, dtype=np.float64) for k, v in inp.items()}
    x = inp["x"].reshape(B, C, S)
    xp = inp["x_prev"].reshape(B, C, S)
    xn = inp["x_next"].reshape(B, C, S)

    bn1s_full = inp["bn1g"] / np.sqrt(inp["bn1v"] + EPS)
    bn1b_full = inp["bn1b"] - inp["bn1m"] * bn1s_full
    bn2s_full = inp["bn2g"] / np.sqrt(inp["bn2v"] + EPS)
    bn2b_full = inp["bn2b"] - inp["bn2m"] * bn2s_full

    per_g = []
    for g in range(4):
        sl = slice(128 * g, 128 * (g + 1))
        wqT = np.stack(
            [
                np.stack([inp["Wq"][i][sl, 128 * k : 128 * (k + 1)].T for k in range(4)])
                for i in range(3)
            ]
        )
        wkT = np.stack(
            [
                np.stack([inp["Wk"][i][sl, 128 * k : 128 * (k + 1)].T for k in range(4)])
                for i in range(3)
            ]
        )
        bqv = np.stack([inp["bq"][i][sl] for i in range(3)], axis=1)
        bkv = np.stack([inp["bk"][i][sl] for i in range(3)], axis=1)

        att_s = np.stack(
            [inp["bng"][i][sl] / np.sqrt(inp["bnv"][i][sl] + EPS) for i in range(3)]
        )  # (3,128)
        xtb = np.stack(
            [
                inp["bnb"][i][sl] + (inp["bo"][i][sl] - inp["bnm"][i][sl]) * att_s[i]
                for i in range(3)
            ]
        )  # (3,128)

        wvo_rows = []
        wobv_row = []
        for i in range(3):
            for hl in range(4):
                hg = 4 * g + hl
                wv_h = inp["Wv"][i][32 * hg : 32 * (hg + 1), :]  # (32, 512)
                bv_h = inp["bv"][i][32 * hg : 32 * (hg + 1)]
                wo_h = inp["Wo"][i, hg]  # (32, 32)
                sc = att_s[i][32 * hl : 32 * (hl + 1)]  # (32,)
                wvo_rows.append(sc[:, None] * (wo_h @ wv_h))
                # fold the (BN-scaled) output bias + BN bias into the u bias:
                # y/denom + xtb == sum_k (u + xtb) p_k / denom
                wobv_row.append(sc * (wo_h @ bv_h) + xtb[i][32 * hl : 32 * (hl + 1)])
        wvo_all = np.concatenate(wvo_rows, axis=0)  # (384, 512)
        wobv = np.concatenate(wobv_row)[None, :]  # (1, 384)
        wvoT = np.stack([wvo_all[:, 128 * k : 128 * (k + 1)].T for k in range(4)])

        c1wT = np.stack(
            [
                np.stack(
                    [
                        inp["c1w"][
                            128 * m : 128 * (m + 1),
                            512 * i + 128 * g : 512 * i + 128 * (g + 1),
                        ]
                        .transpose(1, 2, 3, 0)
                        .reshape(128, 9, 128)
                        for m in range(4)
                    ]
                )
                for i in range(3)
            ]
        )
        c2wT = np.stack(
            [
                inp["c2w"][sl, 128 * k : 128 * (k + 1)]
                .transpose(1, 2, 3, 0)
                .reshape(128, 9, 128)
                for k in range(4)
            ]
        )
        avec = np.concatenate(
            [
                bn1s_full.reshape(4, 128).T,
                bn1b_full.reshape(4, 128).T,
                bn2s_full[sl][:, None],
                bn2b_full[sl][:, None],
            ],
            axis=1,
        )  # (128, 10)

        per_g.append(
            dict(
                wqT=_bf(wqT), wkT=_bf(wkT), wvoT=_bf(wvoT),
                wobv=_f(wobv), c1wT=_bf(c1wT), c2wT=_bf(c2wT),
                dvec=_f(np.concatenate([bqv, bkv], axis=1)),
                avec=_f(avec),
            )
        )

    in_maps = []
    for c in range(NCORES):
        b, g = c // 4, c % 4
        d = dict(per_g[g])
        d["x4"] = _bf(x[b].reshape(4, 128, S))
        d["oth"] = _bf(np.stack([xn[b].reshape(4, 128, S), xp[b].reshape(4, 128, S)]))
        in_maps.append(d)
    return in_maps


_NC_CACHE = {}


def get_nc():
    if "nc" not in _NC_CACHE:
        _NC_CACHE["nc"] = build_nc()
    return _NC_CACHE["nc"]


def assemble(results):
    out = np.zeros((B, C, H, W), dtype=np.float32)
    for c in range(NCORES):
        b, g = c // 4, c % 4
        out[b, 128 * g : 128 * (g + 1)] = results[c]["out"].reshape(128, H, W)
    return out


def kernel(**inputs):
    nc = get_nc()
    in_maps = prepare_core_inputs(inputs)
    res = run_bass_kernel_spmd(nc, in_maps, list(range(NCORES)))
    return assemble(res.results)


# revision 27
# speedup vs baseline: 1.4620x; 1.2253x over previous
"""CSAEncoder Trainium2 kernel: 3-branch cross-attention + concat DoubleConv.

Sharding (8 cores): 2 batch groups x 4 tensor ranks.
Core c: batch b = c // 4, rank g = c % 4.
  - Attention: core computes heads [4g, 4g+4) of all 3 branches for batch b
    (a contiguous 128-channel slab of each branch's output).
  - conv1 computed as partial sums over the core's local 384 input channels
    for ALL 512 output channels; per-branch bf16 AllReduce(add) within the
    4-core batch group (branches 0/1 overlap later attention; branch 2 is
    split into two channel chunks pipelined against h1+conv2).
  - conv2 computed locally: full 512-channel contraction, only the core's own
    128 output channels. No further collective.
Host assembles the full (2, 512, 32, 32) output from the 8 per-core slabs.

v2 changes vs v1:
  - BN bias of each attention branch folded into the v/o projection bias
    host-side (y + b*denom = sum_k (u+b) p), so the post-softmax division is
    a single tensor_mul per head.
  - Softmax denominators: reciprocal_approx_fast (DVE) + gpsimd
    partition_broadcast (SBUF->SBUF) instead of full-precision DVE
    reciprocal + DMA roundtrip through DRAM.
  - y-matmul chains column-packed in pairs (tile_position (0,0)/(0,64)).
  - Per-branch AllReduce in bf16 instead of one fp32 AllReduce at the end.
  - Consolidated input DMAs.
"""

import os
import sys

import ml_dtypes
import numpy as np

for _p in ("/opt/trn_rl_repo",):
    if _p not in sys.path and os.path.isdir(_p):
        sys.path.insert(0, _p)

import concourse.bass as bass
import concourse.mybir as mybir
import concourse.tile as tile
from concourse import bacc
from concourse.bass_utils import run_bass_kernel_spmd

F32 = mybir.dt.float32
BF16 = mybir.dt.bfloat16
AF = mybir.ActivationFunctionType

# The ACT table-set picker is greedy-first-match: with both Exp and Ln in the
# kernel it alternates exp_and_others <-> natural_log (~2.7us per reload, ~25
# reloads).  Restrict matching to the one set that contains every function we
# use (exp, ln, relu, copy) so exactly one table load is emitted.  Keyed by
# name, dict length/order preserved so set ids stay valid.
_ACT_KEEP_SET = "natural_log_exp_and_others"
_orig_get_act_tables = bacc.get_activation_tables


def _patched_get_act_tables(arch):
    tabs = _orig_get_act_tables(arch)
    return {n: (fns if n == _ACT_KEEP_SET else set()) for n, fns in tabs.items()}


bacc.get_activation_tables = _patched_get_act_tables
B, C, H, W, HEADS = 2, 512, 32, 32, 16
D = C // HEADS            # 32
S = H * W                 # 1024
EPS = 1e-5
ISQD = 1.0 / np.sqrt(D)   # folded into the exp activation
NCORES = 8
GROUPS = [[0, 1, 2, 3], [4, 5, 6, 7]]
HP = W + 2                # padded row stride (34)


def build_nc():
    nc = bacc.Bacc(None, target_bir_lowering=False)

    # ---- per-core external inputs -------------------------------------
    x4_d = nc.declare_dram_parameter("x4", [4, 128, S], BF16, isOutput=False)
    oth_d = nc.declare_dram_parameter("oth", [2, 4, 128, S], BF16, isOutput=False)
    wqT_d = nc.declare_dram_parameter("wqT", [3, 4, 128, 128], BF16, isOutput=False)
    wkT_d = nc.declare_dram_parameter("wkT", [3, 4, 128, 128], BF16, isOutput=False)
    wvoT_d = nc.declare_dram_parameter("wvoT", [4, 128, 384], BF16, isOutput=False)
    dvec_d = nc.declare_dram_parameter("dvec", [128, 6], F32, isOutput=False)
    wobv_d = nc.declare_dram_parameter("wobv", [1, 384], F32, isOutput=False)
    c1wT_d = nc.declare_dram_parameter("c1wT", [3, 4, 128, 9, 128], BF16, isOutput=False)
    c2wT_d = nc.declare_dram_parameter("c2wT", [4, 128, 9, 128], BF16, isOutput=False)
    avec_d = nc.declare_dram_parameter("avec", [128, 10], F32, isOutput=False)
    out_d = nc.declare_dram_parameter("out", [128, S], F32, isOutput=True)

    with tile.TileContext(nc) as tc:
        import contextlib

        ctx = contextlib.ExitStack()
        with ctx:
            const = ctx.enter_context(tc.tile_pool(name="const", bufs=1))
            kq = ctx.enter_context(tc.tile_pool(name="kq", bufs=1))
            xtp = ctx.enter_context(tc.tile_pool(name="xtp", bufs=1))
            stg = ctx.enter_context(tc.tile_pool(name="stg", bufs=1))
            brp = ctx.enter_context(tc.tile_pool(name="brp", bufs=2))
            rcp = ctx.enter_context(tc.tile_pool(name="rcp", bufs=2))
            scps = ctx.enter_context(tc.tile_pool(name="scps", bufs=2, space="PSUM"))
            yps = ctx.enter_context(tc.tile_pool(name="yps", bufs=1, space="PSUM"))
            smps = ctx.enter_context(tc.tile_pool(name="smps", bufs=2, space="PSUM"))
            dram = ctx.enter_context(tc.tile_pool(name="dram", bufs=1, space="DRAM"))
            dramw = ctx.enter_context(tc.tile_pool(name="dramw", bufs=4, space="DRAM"))

            # ---- activations + branch-0 weights first (DMA priority) -----
            x_sb = const.tile([128, 4, S], BF16)
            nc.sync.dma_start(out=x_sb, in_=x4_d[:].rearrange("k p s -> p k s"))
            wq_sb = const.tile([128, 3, 4, 128], BF16)
            wk_sb = const.tile([128, 3, 4, 128], BF16)
            nc.sync.dma_start(out=wk_sb, in_=wkT_d[:].rearrange("i k p f -> p i k f"))
            nc.sync.dma_start(out=wq_sb, in_=wqT_d[:].rearrange("i k p f -> p i k f"))
            othp = ctx.enter_context(tc.tile_pool(name="othp", bufs=1))
            oth1 = othp.tile([128, 4, S], BF16, name="oth")
            nc.sync.dma_start(out=oth1, in_=oth_d[0].rearrange("k p s -> p k s"))
            wvo_sb = const.tile([128, 4, 384], BF16)
            nc.sync.dma_start(out=wvo_sb, in_=wvoT_d[:].rearrange("k p f -> p k f"))

            # Small consts: DMA to staging, then re-own on the consuming
            # engine (DVE / ACT) so consumers need no cross-engine const wait.
            dvec_st = const.tile([128, 6], F32)
            nc.gpsimd.dma_start(out=dvec_st, in_=dvec_d[:])
            wobv_st = const.tile([128, 384], F32)
            nc.gpsimd.dma_start(out=wobv_st, in_=wobv_d[:].partition_broadcast(128))
            avec_st = const.tile([128, 10], F32)
            nc.gpsimd.dma_start(out=avec_st, in_=avec_d[:])
            dvec = const.tile([128, 6], F32)
            nc.vector.tensor_copy(dvec, dvec_st)
            wobv_sb = const.tile([128, 384], F32)
            nc.vector.tensor_copy(wobv_sb, wobv_st)
            avec = const.tile([128, 10], F32)
            nc.scalar.activation(out=avec, in_=avec_st, func=AF.Copy)
            bqv_sb = dvec[:, 0:3]
            bkv_sb = dvec[:, 3:6]

            # xt (attention output) slabs + h1 slabs, zero-padded 34x34
            xt_sl = []
            for i in range(3):
                t = xtp.tile([128, HP, HP], BF16, name=f"xt{i}")
                nc.vector.memset(t, 0.0)
                xt_sl.append(t)
            h1_sl = []
            for k in range(4):
                t = xtp.tile([128, HP, HP], BF16, name=f"h1{k}")
                nc.vector.memset(t, 0.0)
                h1_sl.append(t)
            # per-branch conv1 partials (f32 staging for the AllReduce);
            # 2 rotating slots — branch i is shipped before i+1 is written
            brst = {}

            def get_brst(i):
                if i not in brst:
                    brst[i] = brp.tile([128, 4, S], BF16, name="brst")
                return brst[i]

            # Semaphore warmers: absorb const-DMA + memset waits into each
            # engine's observed clock so later compute ops need <=1 wait.
            warm = const.tile([128, 1], F32)
            nc.vector.tensor_copy(warm, dvec[:, 0:1])
            warm2 = const.tile([128, 1], F32)
            nc.scalar.activation(out=warm2, in_=warm, func=AF.Copy)

            # k/q per branch (with biases added), uT tiles
            k_sb = kq.tile([128, 3, S], BF16)
            q_sb = kq.tile([128, 3, S], BF16)
            uT = [kq.tile([128, 3, 4, 33], BF16, name=f"uT{t}") for t in range(8)]

            # ---- projections (k0/q0 upfront; rest fill branch-0 slots) ---
            # x_prev reuses x_next's slot once q_proj(0) has consumed it
            qsrc = [oth1, x_sb, None]

            def load_oth2():
                oth2 = othp.tile([128, 4, S], BF16, name="oth")
                nc.sync.dma_start(out=oth2, in_=oth_d[1].rearrange("k p s -> p k s"))
                qsrc[2] = oth2

            def k_proj(i):
                k_ps = scps.tile([128, S], F32, name="kq_ps", tag="sc")
                for s in range(2):
                    for ks in range(4):
                        nc.tensor.matmul(
                            k_ps[:, 512 * s : 512 * (s + 1)],
                            lhsT=wk_sb[:, i, ks, :],
                            rhs=x_sb[:, ks, 512 * s : 512 * (s + 1)],
                            start=(ks == 0),
                            stop=(ks == 3),
                        )
                nc.vector.tensor_scalar_add(k_sb[:, i, :], k_ps, bkv_sb[:, i : i + 1])

            def q_proj(i):
                q_ps = scps.tile([128, S], F32, name="kq_ps2", tag="sc")
                for s in range(2):
                    for ks in range(4):
                        nc.tensor.matmul(
                            q_ps[:, 512 * s : 512 * (s + 1)],
                            lhsT=wq_sb[:, i, ks, :],
                            rhs=qsrc[i][:, ks, 512 * s : 512 * (s + 1)],
                            start=(ks == 0),
                            stop=(ks == 3),
                        )
                nc.vector.tensor_scalar_add(q_sb[:, i, :], q_ps, bqv_sb[:, i : i + 1])

            def u_proj(t):
                u_ps = smps.tile([128, 384], F32, name="u_ps", tag="sm")
                for ks in range(4):
                    nc.tensor.matmul(
                        u_ps,
                        lhsT=x_sb[:, ks, 128 * t : 128 * (t + 1)],
                        rhs=wvo_sb[:, ks, :],
                        start=(ks == 0),
                        stop=(ks == 3),
                    )
                nc.vector.memset(uT[t][:, :, :, 32:33], 1.0)
                # wobv has the attention-BN bias folded in host-side
                nc.vector.tensor_add(
                    uT[t][:, :, :, 0:32],
                    u_ps.rearrange("p (i h d) -> p i h d", i=3, h=4),
                    wobv_sb.rearrange("p (i h d) -> p i h d", i=3, h=4),
                )

            k_proj(0)
            q_proj(0)
            load_oth2()

            # ---- conv weights (emitted after proj psum freed) ------------
            convw = ctx.enter_context(tc.tile_pool(name="convw", bufs=1))
            pt = ctx.enter_context(tc.tile_pool(name="pt", bufs=16))
            c1w_sb = [
                [convw.tile([128, 9, 128], BF16, name=f"c1w{i}_{m}") for m in range(4)]
                for i in range(3)
            ]
            for i in range(3):
                for m in range(4):
                    nc.sync.dma_start(out=c1w_sb[i][m], in_=c1wT_d[i, m])
            c2w_sb = [convw.tile([128, 9, 128], BF16, name=f"c2w{k}") for k in range(4)]
            for k in range(4):
                nc.sync.dma_start(out=c2w_sb[k], in_=c2wT_d[k])

            def conv1_block(i, m, n):
                """Partial conv1 for xt slab i, out m-tile, spatial half n,
                written (bf16) into brst[i]."""
                ps = smps.tile([128, 512], F32, name="cv", tag="sm")
                for dy in range(3):
                    for dx in range(3):
                        nc.tensor.matmul(
                            ps,
                            lhsT=c1w_sb[i][m][:, dy * 3 + dx, :],
                            rhs=xt_sl[i][:, 16 * n + dy : 16 * n + dy + 16, dx : dx + 32],
                            start=(dy == 0 and dx == 0),
                            stop=(dy == 2 and dx == 2),
                        )
                nc.vector.tensor_copy(get_brst(i)[:, m, 512 * n : 512 * (n + 1)], ps)
                if n == 1:
                    if i < 2:
                        dst = partial[i][128 * m : 128 * (m + 1), :]
                    else:
                        dst = partial2[m // 2][128 * (m % 2) : 128 * (m % 2) + 128, :]
                    nc.sync.dma_start(out=dst, in_=get_brst(i)[:, m, :])

            def attention(i, pr, filler):
                """Heads (2pr, 2pr+1) of branch i.  `filler` is a list of
                thunks (conv1 blocks / projections) sprinkled between the
                per-t score groups to keep PE dense while ACT grinds exps."""
                heads = (2 * pr, 2 * pr + 1)
                pts = {}
                fi = 0
                for t in range(8):
                    for h in heads:
                        sc = scps.tile([128, S], F32, name="sc", tag="sc")
                        p0 = 32 * h
                        for s in range(2):
                            nc.tensor.matmul(
                                sc[:, 512 * s : 512 * (s + 1)],
                                lhsT=k_sb[p0 : p0 + 32, i, 128 * t : 128 * (t + 1)],
                                rhs=q_sb[p0 : p0 + 32, i, 512 * s : 512 * (s + 1)],
                                start=True,
                                stop=True,
                                tile_position=(p0, 0),
                            )
                        ptt = pt.tile([128, S], BF16, name="ptt")
                        nc.scalar.activation(
                            out=ptt, in_=sc, func=AF.Exp, scale=float(ISQD)
                        )
                        pts[(h, t)] = ptt
                    while fi < len(filler) * (t + 1) // 8:
                        filler[fi]()
                        fi += 1
                # y chains: per head, the two query-half chains target the
                # two zero regions of one [33, S] psum tile sequentially
                for h in heads:
                    y2 = yps.tile([33, S], F32, name="y2", tag="y")
                    for s in range(2):
                        for t in range(8):
                            nc.tensor.matmul(
                                y2[:, 512 * s : 512 * (s + 1)],
                                lhsT=uT[t][:, i, h, :],
                                rhs=pts[(h, t)][:, 512 * s : 512 * (s + 1)],
                                start=(t == 0),
                                stop=(t == 7),
                            )
                    p0 = 32 * h
                    # 1/denom as exp(-ln(denom)) on ACT: ln and exp share the
                    # natural_log_exp_and_others table set (no reload), and
                    # both DVE reciprocal (6.5ns/elem) and the custom-DVE
                    # approx op (garbage on this HW) are avoided.
                    rc = rcp.tile([1, S], F32, name="rc")
                    nc.scalar.activation(out=rc, in_=y2[32:33, :], func=AF.Ln)
                    nc.scalar.activation(out=rc, in_=rc, func=AF.Exp, scale=-1.0)
                    rcb = rcp.tile([32, S], F32, name="rcb")
                    nc.gpsimd.partition_broadcast(rcb, rc[:])
                    nc.vector.tensor_mul(
                        xt_sl[i][p0 : p0 + 32, 1:33, 1:33],
                        y2[0:32, :].rearrange("p (a b) -> p a b", b=32),
                        rcb.rearrange("p (a b) -> p a b", b=32),
                    )
                while fi < len(filler):
                    filler[fi]()
                    fi += 1

            # ---- collectives ---------------------------------------------
            # branch 0/1: one [512, S] bf16 AllReduce each, overlapped with
            # the next branch's attention.  branch 2: two [256, S] chunks
            # (m01 / m23) pipelined against h1+conv2.
            partial = [dram.tile([512, S], BF16, name=f"part{i}") for i in range(2)]
            art = [dram.tile([512, S], BF16, name=f"art{i}") for i in range(2)]
            partial2 = [dram.tile([256, S], BF16, name=f"part2{a}") for a in range(2)]
            art2 = [dram.tile([256, S], BF16, name=f"art2{a}") for a in range(2)]

            def ar_branch(i):
                nc.gpsimd.collective_compute(
                    "AllReduce",
                    mybir.AluOpType.add,
                    replica_groups=GROUPS,
                    ins=[partial[i][:]],
                    outs=[art[i][:]],
                )

            def ar2_chunk(a):
                nc.gpsimd.collective_compute(
                    "AllReduce",
                    mybir.AluOpType.add,
                    replica_groups=GROUPS,
                    ins=[partial2[a][:]],
                    outs=[art2[a][:]],
                )

            def conv1_and_ship(i):
                return [
                    (lambda m=m, n=n: conv1_block(i, m, n))
                    for m in range(4)
                    for n in range(2)
                ]

            # ---- phase A: attention with projections/conv1 interleaved ---
            attention(0, 0, [lambda t=t: u_proj(t) for t in range(8)])
            attention(0, 1, [lambda: k_proj(1), lambda: q_proj(1),
                             lambda: k_proj(2), lambda: q_proj(2)])
            f0 = conv1_and_ship(0)
            attention(1, 0, f0[:4])
            attention(1, 1, f0[4:] + [lambda: ar_branch(0)])
            f1 = conv1_and_ship(1)
            attention(2, 0, f1[:4])
            attention(2, 1, f1[4:] + [lambda: ar_branch(1)])
            # branch 2 conv1 + chunked AR at the end
            for m in range(4):
                for n in range(2):
                    conv1_block(2, m, n)
                if m == 1:
                    ar2_chunk(0)
            ar2_chunk(1)

            # ---- phase C: combine, BN1+relu, conv2, BN2+relu, out --------
            arr01 = stg.tile([128, 4, S], BF16, name="arr01", bufs=1)
            arrt = [stg.tile([128, 2, S], BF16, name=f"arrt{j}", bufs=1) for j in range(2)]
            # art0 + art1 (during late attention / AR2 flight)
            nc.sync.dma_start(
                out=arr01, in_=art[0][:].rearrange("(m p) s -> p m s", p=128)
            )
            nc.sync.dma_start(
                out=arrt[0],
                in_=art[1][:].rearrange("(m p) s -> p m s", p=128)[:, 0:2, :],
            )
            nc.sync.dma_start(
                out=arrt[1],
                in_=art[1][:].rearrange("(m p) s -> p m s", p=128)[:, 2:4, :],
            )
            nc.vector.tensor_add(
                arr01[:, 0:2, :], arr01[:, 0:2, :], arrt[0][:, 0:2, :]
            )
            nc.vector.tensor_add(
                arr01[:, 2:4, :], arr01[:, 2:4, :], arrt[1][:, 0:2, :]
            )

            oout = stg.tile([128, S], F32, name="oout", bufs=1)
            ps2 = [smps.tile([128, 512], F32, name=f"cv2_{n}", tag="sm") for n in range(2)]

            def h1_chunk(a):
                """arr01[m01/m23] + art2 chunk -> BN1+relu -> h1 slabs."""
                nc.sync.dma_start(
                    out=arrt[0],
                    in_=art2[a][:].rearrange("(m p) s -> p m s", p=128),
                )
                src = arrt[0]
                nc.vector.tensor_add(
                    src[:, 0:2, :], src[:, 0:2, :], arr01[:, 2 * a : 2 * a + 2, :]
                )
                for j, k in enumerate((2 * a, 2 * a + 1)):
                    nc.scalar.activation(
                        out=h1_sl[k][:, 1:33, 1:33],
                        in_=src[:, j, :].rearrange("p (a b) -> p a b", b=32),
                        func=AF.Relu,
                        bias=avec[:, 4 + k : 5 + k],
                        scale=avec[:, k : k + 1],
                    )

            def conv2_half(a):
                # accumulate k-slabs 2a, 2a+1 into both spatial halves
                for n in range(2):
                    for k in (2 * a, 2 * a + 1):
                        for dy in range(3):
                            for dx in range(3):
                                nc.tensor.matmul(
                                    ps2[n],
                                    lhsT=c2w_sb[k][:, dy * 3 + dx, :],
                                    rhs=h1_sl[k][
                                        :, 16 * n + dy : 16 * n + dy + 16, dx : dx + 32
                                    ],
                                    start=(k == 0 and dy == 0 and dx == 0),
                                    stop=(k == 3 and dy == 2 and dx == 2),
                                )

            h1_chunk(0)
            conv2_half(0)   # overlaps AR2 chunk 1
            h1_chunk(1)
            conv2_half(1)
            for n in range(2):
                nc.scalar.activation(
                    out=oout[:, 512 * n : 512 * (n + 1)],
                    in_=ps2[n],
                    func=AF.Relu,
                    bias=avec[:, 9:10],
                    scale=avec[:, 8:9],
                )
                nc.sync.dma_start(
                    out=out_d[:, 512 * n : 512 * (n + 1)],
                    in_=oout[:, 512 * n : 512 * (n + 1)],
                )

    nc.finalize()
    return nc


def _f(x):
    return np.ascontiguousarray(x, dtype=np.float32)


def _bf(x):
    return np.ascontiguousarray(np.asarray(x, dtype=np.float32).astype(ml_dtypes.bfloat16))


def prepare_core_inputs(inp):
    """Build the 8 per-core input dicts from the full-problem inputs."""
    inp = {k: np.asarray(v, dtype=np.float64) for k, v in inp.items()}
    x = inp["x"].reshape(B, C, S)
    xp = inp["x_prev"].reshape(B, C, S)
    xn = inp["x_next"].reshape(B, C, S)

    bn1s_full = inp["bn1g"] / np.sqrt(inp["bn1v"] + EPS)
    bn1b_full = inp["bn1b"] - inp["bn1m"] * bn1s_full
    bn2s_full = inp["bn2g"] / np.sqrt(inp["bn2v"] + EPS)
    bn2b_full = inp["bn2b"] - inp["bn2m"] * bn2s_full

    per_g = []
    for g in range(4):
        sl = slice(128 * g, 128 * (g + 1))
        wqT = np.stack(
            [
                np.stack([inp["Wq"][i][sl, 128 * k : 128 * (k + 1)].T for k in range(4)])
                for i in range(3)
            ]
        )
        wkT = np.stack(
            [
                np.stack([inp["Wk"][i][sl, 128 * k : 128 * (k + 1)].T for k in range(4)])
                for i in range(3)
            ]
        )
        bqv = np.stack([inp["bq"][i][sl] for i in range(3)], axis=1)
        bkv = np.stack([inp["bk"][i][sl] for i in range(3)], axis=1)

        att_s = np.stack(
            [inp["bng"][i][sl] / np.sqrt(inp["bnv"][i][sl] + EPS) for i in range(3)]
        )  # (3,128)
        xtb = np.stack(
            [
                inp["bnb"][i][sl] + (inp["bo"][i][sl] - inp["bnm"][i][sl]) * att_s[i]
                for i in range(3)
            ]
        )  # (3,128)

        wvo_rows = []
        wobv_row = []
        for i in range(3):
            for hl in range(4):
                hg = 4 * g + hl
                wv_h = inp["Wv"][i][32 * hg : 32 * (hg + 1), :]  # (32, 512)
                bv_h = inp["bv"][i][32 * hg : 32 * (hg + 1)]
                wo_h = inp["Wo"][i, hg]  # (32, 32)
                sc = att_s[i][32 * hl : 32 * (hl + 1)]  # (32,)
                wvo_rows.append(sc[:, None] * (wo_h @ wv_h))
                # fold the (BN-scaled) output bias + BN bias into the u bias:
                # y/denom + xtb == sum_k (u + xtb) p_k / denom
                wobv_row.append(sc * (wo_h @ bv_h) + xtb[i][32 * hl : 32 * (hl + 1)])
        wvo_all = np.concatenate(wvo_rows, axis=0)  # (384, 512)
        wobv = np.concatenate(wobv_row)[None, :]  # (1, 384)
        wvoT = np.stack([wvo_all[:, 128 * k : 128 * (k + 1)].T for k in range(4)])

        c1wT = np.stack(
            [
                np.stack(
                    [
                        inp["c1w"][
                            128 * m : 128 * (m + 1),
                            512 * i + 128 * g : 512 * i + 128 * (g + 1),
                        ]
                        .transpose(1, 2, 3, 0)
                        .reshape(128, 9, 128)
                        for m in range(4)
                    ]
                )
                for i in range(3)
            ]
        )
        c2wT = np.stack(
            [
                inp["c2w"][sl, 128 * k : 128 * (k + 1)]
                .transpose(1, 2, 3, 0)
                .reshape(128, 9, 128)
                for k in range(4)
            ]
        )
        avec = np.concatenate(
            [
                bn1s_full.reshape(4, 128).T,
                bn1b_full.reshape(4, 128).T,
                bn2s_full[sl][:, None],
                bn2b_full[sl][:, None],
            ],
            axis=1,
        )  # (128, 10)

        per_g.append(
            dict(
                wqT=_bf(wqT), wkT=_bf(wkT), wvoT=_bf(wvoT),
                wobv=_f(wobv), c1wT=_bf(c1wT), c2wT=_bf(c2wT),
                dvec=_f(np.concatenate([bqv, bkv], axis=1)),
                avec=_f(avec),
            )
        )

    in_maps = []
    for c in range(NCORES):
        b, g = c // 4, c % 4
        d = dict(per_g[g])
        d["x4"] = _bf(x[b].reshape(4, 128, S))
        d["oth"] = _bf(np.stack([xn[b].reshape(4, 128, S), xp[b].reshape(4, 128, S)]))
        in_maps.append(d)
    return in_maps


_NC_CACHE = {}


def get_nc():
    if "nc" not in _NC_CACHE:
        _NC_CACHE["nc"] = build_nc()
    return _NC_CACHE["nc"]


def assemble(results):
    out = np.zeros((B, C, H, W), dtype=np.float32)
    for c in range(NCORES):
        b, g = c // 4, c % 4
        out[b, 128 * g : 128 * (g + 1)] = results[c]["out"].reshape(128, H, W)
    return out


def kernel(**inputs):
    nc = get_nc()
    in_maps = prepare_core_inputs(inputs)
    res = run_bass_kernel_spmd(nc, in_maps, list(range(NCORES)))
    return assemble(res.results)


# revision 29
# speedup vs baseline: 1.4641x; 1.0015x over previous
"""CSAEncoder Trainium2 kernel: 3-branch cross-attention + concat DoubleConv.

Sharding (8 cores): 2 batch groups x 4 tensor ranks.
Core c: batch b = c // 4, rank g = c % 4.
  - Attention: core computes heads [4g, 4g+4) of all 3 branches for batch b
    (a contiguous 128-channel slab of each branch's output).
  - conv1 computed as partial sums over the core's local 384 input channels
    for ALL 512 output channels; per-branch bf16 AllReduce(add) within the
    4-core batch group (branches 0/1 overlap later attention; branch 2 is
    split into two channel chunks pipelined against h1+conv2).
  - conv2 computed locally: full 512-channel contraction, only the core's own
    128 output channels. No further collective.
Host assembles the full (2, 512, 32, 32) output from the 8 per-core slabs.

v2 changes vs v1:
  - BN bias of each attention branch folded into the v/o projection bias
    host-side (y + b*denom = sum_k (u+b) p), so the post-softmax division is
    a single tensor_mul per head.
  - Softmax denominators: reciprocal_approx_fast (DVE) + gpsimd
    partition_broadcast (SBUF->SBUF) instead of full-precision DVE
    reciprocal + DMA roundtrip through DRAM.
  - y-matmul chains column-packed in pairs (tile_position (0,0)/(0,64)).
  - Per-branch AllReduce in bf16 instead of one fp32 AllReduce at the end.
  - Consolidated input DMAs.
"""

import os
import sys

import ml_dtypes
import numpy as np

for _p in ("/opt/trn_rl_repo",):
    if _p not in sys.path and os.path.isdir(_p):
        sys.path.insert(0, _p)

import concourse.bass as bass
import concourse.mybir as mybir
import concourse.tile as tile
from concourse import bacc
from concourse.bass_utils import run_bass_kernel_spmd

F32 = mybir.dt.float32
BF16 = mybir.dt.bfloat16
AF = mybir.ActivationFunctionType

# The ACT table-set picker is greedy-first-match: with both Exp and Ln in the
# kernel it alternates exp_and_others <-> natural_log (~2.7us per reload, ~25
# reloads).  Restrict matching to the one set that contains every function we
# use (exp, ln, relu, copy) so exactly one table load is emitted.  Keyed by
# name, dict length/order preserved so set ids stay valid.
_ACT_KEEP_SET = "natural_log_exp_and_others"
_orig_get_act_tables = bacc.get_activation_tables


def _patched_get_act_tables(arch):
    tabs = _orig_get_act_tables(arch)
    return {n: (fns if n == _ACT_KEEP_SET else set()) for n, fns in tabs.items()}


bacc.get_activation_tables = _patched_get_act_tables
B, C, H, W, HEADS = 2, 512, 32, 32, 16
D = C // HEADS            # 32
S = H * W                 # 1024
EPS = 1e-5
ISQD = 1.0 / np.sqrt(D)   # folded into the exp activation
NCORES = 8
GROUPS = [[0, 1, 2, 3], [4, 5, 6, 7]]
HP = W + 2                # padded row stride (34)


def build_nc():
    nc = bacc.Bacc(None, target_bir_lowering=False)

    # ---- per-core external inputs -------------------------------------
    x4_d = nc.declare_dram_parameter("x4", [4, 128, S], BF16, isOutput=False)
    oth_d = nc.declare_dram_parameter("oth", [2, 4, 128, S], BF16, isOutput=False)
    wqT_d = nc.declare_dram_parameter("wqT", [3, 4, 128, 128], BF16, isOutput=False)
    wkT_d = nc.declare_dram_parameter("wkT", [3, 4, 128, 128], BF16, isOutput=False)
    wvoT_d = nc.declare_dram_parameter("wvoT", [4, 128, 384], BF16, isOutput=False)
    dvec_d = nc.declare_dram_parameter("dvec", [128, 6], F32, isOutput=False)
    wobv_d = nc.declare_dram_parameter("wobv", [1, 384], F32, isOutput=False)
    c1wT_d = nc.declare_dram_parameter("c1wT", [3, 4, 128, 9, 128], BF16, isOutput=False)
    c2wT_d = nc.declare_dram_parameter("c2wT", [4, 128, 9, 128], BF16, isOutput=False)
    avec_d = nc.declare_dram_parameter("avec", [128, 10], F32, isOutput=False)
    out_d = nc.declare_dram_parameter("out", [128, S], F32, isOutput=True)

    with tile.TileContext(nc) as tc:
        import contextlib

        ctx = contextlib.ExitStack()
        with ctx:
            const = ctx.enter_context(tc.tile_pool(name="const", bufs=1))
            kq = ctx.enter_context(tc.tile_pool(name="kq", bufs=1))
            xtp = ctx.enter_context(tc.tile_pool(name="xtp", bufs=1))
            stg = ctx.enter_context(tc.tile_pool(name="stg", bufs=1))
            brp = ctx.enter_context(tc.tile_pool(name="brp", bufs=2))
            rcp = ctx.enter_context(tc.tile_pool(name="rcp", bufs=2))
            scps = ctx.enter_context(tc.tile_pool(name="scps", bufs=2, space="PSUM"))
            yps = ctx.enter_context(tc.tile_pool(name="yps", bufs=1, space="PSUM"))
            smps = ctx.enter_context(tc.tile_pool(name="smps", bufs=2, space="PSUM"))
            dram = ctx.enter_context(tc.tile_pool(name="dram", bufs=1, space="DRAM"))
            dramw = ctx.enter_context(tc.tile_pool(name="dramw", bufs=4, space="DRAM"))

            # ---- activations + branch-0 weights first (DMA priority) -----
            x_sb = const.tile([128, 4, S], BF16)
            nc.sync.dma_start(out=x_sb, in_=x4_d[:].rearrange("k p s -> p k s"))
            wq_sb = const.tile([128, 3, 4, 128], BF16)
            wk_sb = const.tile([128, 3, 4, 128], BF16)
            nc.sync.dma_start(out=wk_sb, in_=wkT_d[:].rearrange("i k p f -> p i k f"))
            nc.sync.dma_start(out=wq_sb, in_=wqT_d[:].rearrange("i k p f -> p i k f"))
            othp = ctx.enter_context(tc.tile_pool(name="othp", bufs=1))
            oth1 = othp.tile([128, 4, S], BF16, name="oth")
            nc.sync.dma_start(out=oth1, in_=oth_d[0].rearrange("k p s -> p k s"))
            wvo_sb = const.tile([128, 4, 384], BF16)
            nc.sync.dma_start(out=wvo_sb, in_=wvoT_d[:].rearrange("k p f -> p k f"))

            # Small consts: DMA to staging, then re-own on the consuming
            # engine (DVE / ACT) so consumers need no cross-engine const wait.
            dvec_st = const.tile([128, 6], F32)
            nc.gpsimd.dma_start(out=dvec_st, in_=dvec_d[:])
            wobv_st = const.tile([128, 384], F32)
            nc.gpsimd.dma_start(out=wobv_st, in_=wobv_d[:].partition_broadcast(128))
            avec_st = const.tile([128, 10], F32)
            nc.gpsimd.dma_start(out=avec_st, in_=avec_d[:])
            dvec = const.tile([128, 6], F32)
            nc.vector.tensor_copy(dvec, dvec_st)
            wobv_sb = const.tile([128, 384], F32)
            nc.vector.tensor_copy(wobv_sb, wobv_st)
            avec = const.tile([128, 10], F32)
            nc.scalar.activation(out=avec, in_=avec_st, func=AF.Copy)
            bqv_sb = dvec[:, 0:3]
            bkv_sb = dvec[:, 3:6]

            # xt (attention output) slabs + h1 slabs, zero-padded 34x34
            xt_sl = []
            for i in range(3):
                t = xtp.tile([128, HP, HP], BF16, name=f"xt{i}")
                nc.vector.memset(t, 0.0)
                xt_sl.append(t)
            h1_sl = []
            for k in range(4):
                t = xtp.tile([128, HP, HP], BF16, name=f"h1{k}")
                nc.vector.memset(t, 0.0)
                h1_sl.append(t)
            # per-branch conv1 partials (f32 staging for the AllReduce);
            # 2 rotating slots — branch i is shipped before i+1 is written
            brst = {}

            def get_brst(i):
                if i not in brst:
                    brst[i] = brp.tile([128, 4, S], BF16, name="brst")
                return brst[i]

            # Semaphore warmers: absorb const-DMA + memset waits into each
            # engine's observed clock so later compute ops need <=1 wait.
            warm = const.tile([128, 1], F32)
            nc.vector.tensor_copy(warm, dvec[:, 0:1])
            warm2 = const.tile([128, 1], F32)
            nc.scalar.activation(out=warm2, in_=warm, func=AF.Copy)

            # k/q per branch (with biases added), uT tiles
            k_sb = kq.tile([128, 3, S], BF16)
            q_sb = kq.tile([128, 3, S], BF16)
            uT = [kq.tile([128, 3, 4, 33], BF16, name=f"uT{t}") for t in range(8)]

            # ---- projections (k0/q0 upfront; rest fill branch-0 slots) ---
            # x_prev reuses x_next's slot once q_proj(0) has consumed it
            qsrc = [oth1, x_sb, None]

            def load_oth2():
                oth2 = othp.tile([128, 4, S], BF16, name="oth")
                nc.sync.dma_start(out=oth2, in_=oth_d[1].rearrange("k p s -> p k s"))
                qsrc[2] = oth2

            def k_proj(i):
                k_ps = scps.tile([128, S], F32, name="kq_ps", tag="sc")
                for s in range(2):
                    for ks in range(4):
                        nc.tensor.matmul(
                            k_ps[:, 512 * s : 512 * (s + 1)],
                            lhsT=wk_sb[:, i, ks, :],
                            rhs=x_sb[:, ks, 512 * s : 512 * (s + 1)],
                            start=(ks == 0),
                            stop=(ks == 3),
                        )
                nc.vector.tensor_scalar_add(k_sb[:, i, :], k_ps, bkv_sb[:, i : i + 1])

            def q_proj(i):
                q_ps = scps.tile([128, S], F32, name="kq_ps2", tag="sc")
                for s in range(2):
                    for ks in range(4):
                        nc.tensor.matmul(
                            q_ps[:, 512 * s : 512 * (s + 1)],
                            lhsT=wq_sb[:, i, ks, :],
                            rhs=qsrc[i][:, ks, 512 * s : 512 * (s + 1)],
                            start=(ks == 0),
                            stop=(ks == 3),
                        )
                nc.vector.tensor_scalar_add(q_sb[:, i, :], q_ps, bqv_sb[:, i : i + 1])

            def u_proj(t):
                u_ps = smps.tile([128, 384], F32, name="u_ps", tag="sm")
                for ks in range(4):
                    nc.tensor.matmul(
                        u_ps,
                        lhsT=x_sb[:, ks, 128 * t : 128 * (t + 1)],
                        rhs=wvo_sb[:, ks, :],
                        start=(ks == 0),
                        stop=(ks == 3),
                    )
                nc.vector.memset(uT[t][:, :, :, 32:33], 1.0)
                # wobv has the attention-BN bias folded in host-side
                nc.vector.tensor_add(
                    uT[t][:, :, :, 0:32],
                    u_ps.rearrange("p (i h d) -> p i h d", i=3, h=4),
                    wobv_sb.rearrange("p (i h d) -> p i h d", i=3, h=4),
                )

            k_proj(0)
            q_proj(0)
            load_oth2()

            # ---- conv weights (emitted after proj psum freed) ------------
            convw = ctx.enter_context(tc.tile_pool(name="convw", bufs=1))
            pt = ctx.enter_context(tc.tile_pool(name="pt", bufs=16))
            c1w_sb = [
                [convw.tile([128, 9, 128], BF16, name=f"c1w{i}_{m}") for m in range(4)]
                for i in range(3)
            ]
            for i in range(3):
                for m in range(4):
                    nc.sync.dma_start(out=c1w_sb[i][m], in_=c1wT_d[i, m])
            c2w_sb = [convw.tile([128, 9, 128], BF16, name=f"c2w{k}") for k in range(4)]
            for k in range(4):
                nc.sync.dma_start(out=c2w_sb[k], in_=c2wT_d[k])

            def conv1_block(i, m, n):
                """Partial conv1 for xt slab i, out m-tile, spatial half n,
                written (bf16) into brst[i]."""
                ps = smps.tile([128, 512], F32, name="cv", tag="sm")
                for dy in range(3):
                    for dx in range(3):
                        nc.tensor.matmul(
                            ps,
                            lhsT=c1w_sb[i][m][:, dy * 3 + dx, :],
                            rhs=xt_sl[i][:, 16 * n + dy : 16 * n + dy + 16, dx : dx + 32],
                            start=(dy == 0 and dx == 0),
                            stop=(dy == 2 and dx == 2),
                        )
                nc.vector.tensor_copy(get_brst(i)[:, m, 512 * n : 512 * (n + 1)], ps)
                if n == 1:
                    if i < 2:
                        dst = partial[i][128 * m : 128 * (m + 1), :]
                    else:
                        dst = partial2[m // 2][128 * (m % 2) : 128 * (m % 2) + 128, :]
                    nc.gpsimd.dma_start(out=dst, in_=get_brst(i)[:, m, :])

            def attention(i, pr, filler):
                """Heads (2pr, 2pr+1) of branch i.  `filler` is a list of
                thunks (conv1 blocks / projections) sprinkled between the
                per-t score groups to keep PE dense while ACT grinds exps."""
                heads = (2 * pr, 2 * pr + 1)
                pts = {}
                fi = 0
                for t in range(8):
                    for h in heads:
                        sc = scps.tile([128, S], F32, name="sc", tag="sc")
                        p0 = 32 * h
                        for s in range(2):
                            nc.tensor.matmul(
                                sc[:, 512 * s : 512 * (s + 1)],
                                lhsT=k_sb[p0 : p0 + 32, i, 128 * t : 128 * (t + 1)],
                                rhs=q_sb[p0 : p0 + 32, i, 512 * s : 512 * (s + 1)],
                                start=True,
                                stop=True,
                                tile_position=(p0, 0),
                            )
                        ptt = pt.tile([128, S], BF16, name="ptt")
                        nc.scalar.activation(
                            out=ptt, in_=sc, func=AF.Exp, scale=float(ISQD)
                        )
                        pts[(h, t)] = ptt
                    while fi < len(filler) * (t + 1) // 8:
                        filler[fi]()
                        fi += 1
                # y chains: per head, the two query-half chains target the
                # two zero regions of one [33, S] psum tile sequentially
                for h in heads:
                    y2 = yps.tile([33, S], F32, name="y2", tag="y")
                    for s in range(2):
                        for t in range(8):
                            nc.tensor.matmul(
                                y2[:, 512 * s : 512 * (s + 1)],
                                lhsT=uT[t][:, i, h, :],
                                rhs=pts[(h, t)][:, 512 * s : 512 * (s + 1)],
                                start=(t == 0),
                                stop=(t == 7),
                            )
                    p0 = 32 * h
                    # 1/denom as exp(-ln(denom)) on ACT: ln and exp share the
                    # natural_log_exp_and_others table set (no reload), and
                    # both DVE reciprocal (6.5ns/elem) and the custom-DVE
                    # approx op (garbage on this HW) are avoided.
                    rc = rcp.tile([1, S], F32, name="rc")
                    nc.scalar.activation(out=rc, in_=y2[32:33, :], func=AF.Ln)
                    nc.scalar.activation(out=rc, in_=rc, func=AF.Exp, scale=-1.0)
                    rcb = rcp.tile([32, S], F32, name="rcb")
                    nc.gpsimd.partition_broadcast(rcb, rc[:])
                    nc.vector.tensor_mul(
                        xt_sl[i][p0 : p0 + 32, 1:33, 1:33],
                        y2[0:32, :].rearrange("p (a b) -> p a b", b=32),
                        rcb.rearrange("p (a b) -> p a b", b=32),
                    )
                while fi < len(filler):
                    filler[fi]()
                    fi += 1

            # ---- collectives ---------------------------------------------
            # branch 0/1: one [512, S] bf16 AllReduce each, overlapped with
            # the next branch's attention.  branch 2: two [256, S] chunks
            # (m01 / m23) pipelined against h1+conv2.
            partial = [dram.tile([512, S], BF16, name=f"part{i}") for i in range(2)]
            art = [dram.tile([512, S], BF16, name=f"art{i}") for i in range(2)]
            partial2 = [dram.tile([256, S], BF16, name=f"part2{a}") for a in range(2)]
            art2 = [dram.tile([256, S], BF16, name=f"art2{a}") for a in range(2)]

            def ar_branch(i):
                nc.gpsimd.collective_compute(
                    "AllReduce",
                    mybir.AluOpType.add,
                    replica_groups=GROUPS,
                    ins=[partial[i][:]],
                    outs=[art[i][:]],
                )

            def ar2_chunk(a):
                nc.gpsimd.collective_compute(
                    "AllReduce",
                    mybir.AluOpType.add,
                    replica_groups=GROUPS,
                    ins=[partial2[a][:]],
                    outs=[art2[a][:]],
                )

            def conv1_and_ship(i):
                return [
                    (lambda m=m, n=n: conv1_block(i, m, n))
                    for m in range(4)
                    for n in range(2)
                ]

            # ---- phase A: attention with projections/conv1 interleaved ---
            attention(0, 0, [lambda t=t: u_proj(t) for t in range(8)])
            attention(0, 1, [lambda: k_proj(1), lambda: q_proj(1),
                             lambda: k_proj(2), lambda: q_proj(2)])
            f0 = conv1_and_ship(0)
            attention(1, 0, f0[:4])
            attention(1, 1, f0[4:] + [lambda: ar_branch(0)])
            f1 = conv1_and_ship(1)
            attention(2, 0, f1[:4])
            attention(2, 1, f1[4:] + [lambda: ar_branch(1)])
            # branch 2 conv1 + chunked AR at the end
            for m in range(4):
                for n in range(2):
                    conv1_block(2, m, n)
                if m == 1:
                    ar2_chunk(0)
            ar2_chunk(1)

            # ---- phase C: combine, BN1+relu, conv2, BN2+relu, out --------
            arr01 = stg.tile([128, 4, S], BF16, name="arr01", bufs=1)
            arrt = [stg.tile([128, 2, S], BF16, name=f"arrt{j}", bufs=1) for j in range(2)]
            # art0 + art1 (during late attention / AR2 flight)
            nc.sync.dma_start(
                out=arr01, in_=art[0][:].rearrange("(m p) s -> p m s", p=128)
            )
            nc.sync.dma_start(
                out=arrt[0],
                in_=art[1][:].rearrange("(m p) s -> p m s", p=128)[:, 0:2, :],
            )
            nc.sync.dma_start(
                out=arrt[1],
                in_=art[1][:].rearrange("(m p) s -> p m s", p=128)[:, 2:4, :],
            )
            nc.vector.tensor_add(
                arr01[:, 0:2, :], arr01[:, 0:2, :], arrt[0][:, 0:2, :]
            )
            nc.vector.tensor_add(
                arr01[:, 2:4, :], arr01[:, 2:4, :], arrt[1][:, 0:2, :]
            )

            oout = stg.tile([128, S], F32, name="oout", bufs=1)
            ps2 = [smps.tile([128, 512], F32, name=f"cv2_{n}", tag="sm") for n in range(2)]

            def h1_chunk(a):
                """arr01[m01/m23] + art2 chunk -> BN1+relu -> h1 slabs."""
                nc.sync.dma_start(
                    out=arrt[0],
                    in_=art2[a][:].rearrange("(m p) s -> p m s", p=128),
                )
                src = arrt[0]
                nc.vector.tensor_add(
                    src[:, 0:2, :], src[:, 0:2, :], arr01[:, 2 * a : 2 * a + 2, :]
                )
                for j, k in enumerate((2 * a, 2 * a + 1)):
                    nc.scalar.activation(
                        out=h1_sl[k][:, 1:33, 1:33],
                        in_=src[:, j, :].rearrange("p (a b) -> p a b", b=32),
                        func=AF.Relu,
                        bias=avec[:, 4 + k : 5 + k],
                        scale=avec[:, k : k + 1],
                    )

            def conv2_half(a):
                # accumulate k-slabs 2a, 2a+1 into both spatial halves
                for n in range(2):
                    for k in (2 * a, 2 * a + 1):
                        for dy in range(3):
                            for dx in range(3):
                                nc.tensor.matmul(
                                    ps2[n],
                                    lhsT=c2w_sb[k][:, dy * 3 + dx, :],
                                    rhs=h1_sl[k][
                                        :, 16 * n + dy : 16 * n + dy + 16, dx : dx + 32
                                    ],
                                    start=(k == 0 and dy == 0 and dx == 0),
                                    stop=(k == 3 and dy == 2 and dx == 2),
                                )

            h1_chunk(0)
            conv2_half(0)   # overlaps AR2 chunk 1
            h1_chunk(1)
            conv2_half(1)
            for n in range(2):
                nc.scalar.activation(
                    out=oout[:, 512 * n : 512 * (n + 1)],
                    in_=ps2[n],
                    func=AF.Relu,
                    bias=avec[:, 9:10],
                    scale=avec[:, 8:9],
                )
                nc.sync.dma_start(
                    out=out_d[:, 512 * n : 512 * (n + 1)],
                    in_=oout[:, 512 * n : 512 * (n + 1)],
                )

    nc.finalize()
    return nc


def _f(x):
    return np.ascontiguousarray(x, dtype=np.float32)


def _bf(x):
    return np.ascontiguousarray(np.asarray(x, dtype=np.float32).astype(ml_dtypes.bfloat16))


def prepare_core_inputs(inp):
    """Build the 8 per-core input dicts from the full-problem inputs."""
    inp = {k: np.asarray(v, dtype=np.float64) for k, v in inp.items()}
    x = inp["x"].reshape(B, C, S)
    xp = inp["x_prev"].reshape(B, C, S)
    xn = inp["x_next"].reshape(B, C, S)

    bn1s_full = inp["bn1g"] / np.sqrt(inp["bn1v"] + EPS)
    bn1b_full = inp["bn1b"] - inp["bn1m"] * bn1s_full
    bn2s_full = inp["bn2g"] / np.sqrt(inp["bn2v"] + EPS)
    bn2b_full = inp["bn2b"] - inp["bn2m"] * bn2s_full

    per_g = []
    for g in range(4):
        sl = slice(128 * g, 128 * (g + 1))
        wqT = np.stack(
            [
                np.stack([inp["Wq"][i][sl, 128 * k : 128 * (k + 1)].T for k in range(4)])
                for i in range(3)
            ]
        )
        wkT = np.stack(
            [
                np.stack([inp["Wk"][i][sl, 128 * k : 128 * (k + 1)].T for k in range(4)])
                for i in range(3)
            ]
        )
        bqv = np.stack([inp["bq"][i][sl] for i in range(3)], axis=1)
        bkv = np.stack([inp["bk"][i][sl] for i in range(3)], axis=1)

        att_s = np.stack(
            [inp["bng"][i][sl] / np.sqrt(inp["bnv"][i][sl] + EPS) for i in range(3)]
        )  # (3,128)
        xtb = np.stack(
            [
                inp["bnb"][i][sl] + (inp["bo"][i][sl] - inp["bnm"][i][sl]) * att_s[i]
                for i in range(3)
            ]
        )  # (3,128)

        wvo_rows = []
        wobv_row = []
        for i in range(3):
            for hl in range(4):
                hg = 4 * g + hl
                wv_h = inp["Wv"][i][32 * hg : 32 * (hg + 1), :]  # (32, 512)
                bv_h = inp["bv"][i][32 * hg : 32 * (hg + 1)]
                wo_h = inp["Wo"][i, hg]  # (32, 32)
                sc = att_s[i][32 * hl : 32 * (hl + 1)]  # (32,)
                wvo_rows.append(sc[:, None] * (wo_h @ wv_h))
                # fold the (BN-scaled) output bias + BN bias into the u bias:
                # y/denom + xtb == sum_k (u + xtb) p_k / denom
                wobv_row.append(sc * (wo_h @ bv_h) + xtb[i][32 * hl : 32 * (hl + 1)])
        wvo_all = np.concatenate(wvo_rows, axis=0)  # (384, 512)
        wobv = np.concatenate(wobv_row)[None, :]  # (1, 384)
        wvoT = np.stack([wvo_all[:, 128 * k : 128 * (k + 1)].T for k in range(4)])

        c1wT = np.stack(
            [
                np.stack(
                    [
                        inp["c1w"][
                            128 * m : 128 * (m + 1),
                            512 * i + 128 * g : 512 * i + 128 * (g + 1),
                        ]
                        .transpose(1, 2, 3, 0)
                        .reshape(128, 9, 128)
                        for m in range(4)
                    ]
                )
                for i in range(3)
            ]
        )
        c2wT = np.stack(
            [
                inp["c2w"][sl, 128 * k : 128 * (k + 1)]
                .transpose(1, 2, 3, 0)
                .reshape(128, 9, 128)
                for k in range(4)
            ]
        )
        avec = np.concatenate(
            [
                bn1s_full.reshape(4, 128).T,
                bn1b_full.reshape(4, 128).T,
                bn2s_full[sl][:, None],
                bn2b_full[sl][:, None],
            ],
            axis=1,
        )  # (128, 10)

        per_g.append(
            dict(
                wqT=_bf(wqT), wkT=_bf(wkT), wvoT=_bf(wvoT),
                wobv=_f(wobv), c1wT=_bf(c1wT), c2wT=_bf(c2wT),
                dvec=_f(np.concatenate([bqv, bkv], axis=1)),
                avec=_f(avec),
            )
        )

    in_maps = []
    for c in range(NCORES):
        b, g = c // 4, c % 4
        d = dict(per_g[g])
        d["x4"] = _bf(x[b].reshape(4, 128, S))
        d["oth"] = _bf(np.stack([xn[b].reshape(4, 128, S), xp[b].reshape(4, 128, S)]))
        in_maps.append(d)
    return in_maps


_NC_CACHE = {}


def get_nc():
    if "nc" not in _NC_CACHE:
        _NC_CACHE["nc"] = build_nc()
    return _NC_CACHE["nc"]


def assemble(results):
    out = np.zeros((B, C, H, W), dtype=np.float32)
    for c in range(NCORES):
        b, g = c // 4, c % 4
        out[b, 128 * g : 128 * (g + 1)] = results[c]["out"].reshape(128, H, W)
    return out


def kernel(**inputs):
    nc = get_nc()
    in_maps = prepare_core_inputs(inputs)
    res = run_bass_kernel_spmd(nc, in_maps, list(range(NCORES)))
    return assemble(res.results)


# revision 30
# speedup vs baseline: 1.5294x; 1.0446x over previous
"""CSAEncoder Trainium2 kernel: 3-branch cross-attention + concat DoubleConv.

Sharding (8 cores): 2 batch groups x 4 tensor ranks.
Core c: batch b = c // 4, rank g = c % 4.
  - Attention: core computes heads [4g, 4g+4) of all 3 branches for batch b
    (a contiguous 128-channel slab of each branch's output).
  - conv1 computed as partial sums over the core's local 384 input channels
    for ALL 512 output channels; per-branch bf16 AllReduce(add) within the
    4-core batch group (branches 0/1 overlap later attention; branch 2 is
    split into two channel chunks pipelined against h1+conv2).
  - conv2 computed locally: full 512-channel contraction, only the core's own
    128 output channels. No further collective.
Host assembles the full (2, 512, 32, 32) output from the 8 per-core slabs.

v2 changes vs v1:
  - BN bias of each attention branch folded into the v/o projection bias
    host-side (y + b*denom = sum_k (u+b) p), so the post-softmax division is
    a single tensor_mul per head.
  - Softmax denominators: 1/d computed as exp(-ln d) on ACT (ln and exp
    share one activation-table set; see the get_activation_tables patch) +
    gpsimd partition_broadcast (SBUF->SBUF) instead of the 6.5ns/elem DVE
    reciprocal + DMA roundtrip through DRAM.  (The custom-DVE
    reciprocal_approx_fast op produces garbage on this hardware.)
  - Per-branch AllReduce in bf16 instead of one fp32 AllReduce at the end.
  - Consolidated input DMAs; projections interleaved into branch-0 slots.
  - AllReduce-dependent DMAs (art loads) are kept OFF the gpsimd and vector
    engine streams: the Tile scheduler otherwise hoists them ahead of the
    branch-2 softmax broadcasts/muls in the engine FIFO, serializing the
    whole pipeline behind a slow collective.
"""

import os
import sys

import ml_dtypes
import numpy as np

for _p in ("/opt/trn_rl_repo",):
    if _p not in sys.path and os.path.isdir(_p):
        sys.path.insert(0, _p)

import concourse.bass as bass
import concourse.mybir as mybir
import concourse.tile as tile
from concourse import bacc
from concourse.bass_utils import run_bass_kernel_spmd

F32 = mybir.dt.float32
BF16 = mybir.dt.bfloat16
AF = mybir.ActivationFunctionType

# The ACT table-set picker is greedy-first-match: with both Exp and Ln in the
# kernel it alternates exp_and_others <-> natural_log (~2.7us per reload, ~25
# reloads).  Restrict matching to the one set that contains every function we
# use (exp, ln, relu, copy) so exactly one table load is emitted.  Keyed by
# name, dict length/order preserved so set ids stay valid.
_ACT_KEEP_SET = "natural_log_exp_and_others"
_orig_get_act_tables = bacc.get_activation_tables


def _patched_get_act_tables(arch):
    tabs = _orig_get_act_tables(arch)
    return {n: (fns if n == _ACT_KEEP_SET else set()) for n, fns in tabs.items()}


bacc.get_activation_tables = _patched_get_act_tables
B, C, H, W, HEADS = 2, 512, 32, 32, 16
D = C // HEADS            # 32
S = H * W                 # 1024
EPS = 1e-5
ISQD = 1.0 / np.sqrt(D)   # folded into the exp activation
NCORES = 8
GROUPS = [[0, 1, 2, 3], [4, 5, 6, 7]]
HP = W + 2                # padded row stride (34)


def build_nc():
    nc = bacc.Bacc(None, target_bir_lowering=False)

    # ---- per-core external inputs -------------------------------------
    x4_d = nc.declare_dram_parameter("x4", [4, 128, S], BF16, isOutput=False)
    oth_d = nc.declare_dram_parameter("oth", [2, 4, 128, S], BF16, isOutput=False)
    wqT_d = nc.declare_dram_parameter("wqT", [3, 4, 128, 128], BF16, isOutput=False)
    wkT_d = nc.declare_dram_parameter("wkT", [3, 4, 128, 128], BF16, isOutput=False)
    wvoT_d = nc.declare_dram_parameter("wvoT", [4, 128, 384], BF16, isOutput=False)
    dvec_d = nc.declare_dram_parameter("dvec", [128, 6], F32, isOutput=False)
    wobv_d = nc.declare_dram_parameter("wobv", [1, 384], F32, isOutput=False)
    c1wT_d = nc.declare_dram_parameter("c1wT", [3, 4, 128, 9, 128], BF16, isOutput=False)
    c2wT_d = nc.declare_dram_parameter("c2wT", [4, 128, 9, 128], BF16, isOutput=False)
    avec_d = nc.declare_dram_parameter("avec", [128, 10], F32, isOutput=False)
    out_d = nc.declare_dram_parameter("out", [128, S], F32, isOutput=True)

    with tile.TileContext(nc) as tc:
        import contextlib

        ctx = contextlib.ExitStack()
        with ctx:
            const = ctx.enter_context(tc.tile_pool(name="const", bufs=1))
            kq = ctx.enter_context(tc.tile_pool(name="kq", bufs=1))
            xtp = ctx.enter_context(tc.tile_pool(name="xtp", bufs=1))
            stg = ctx.enter_context(tc.tile_pool(name="stg", bufs=1))
            brp = ctx.enter_context(tc.tile_pool(name="brp", bufs=2))
            rcp = ctx.enter_context(tc.tile_pool(name="rcp", bufs=2))
            scps = ctx.enter_context(tc.tile_pool(name="scps", bufs=2, space="PSUM"))
            yps = ctx.enter_context(tc.tile_pool(name="yps", bufs=1, space="PSUM"))
            smps = ctx.enter_context(tc.tile_pool(name="smps", bufs=2, space="PSUM"))
            dram = ctx.enter_context(tc.tile_pool(name="dram", bufs=1, space="DRAM"))

            # ---- activations + branch-0 weights first (DMA priority) -----
            x_sb = const.tile([128, 4, S], BF16)
            nc.sync.dma_start(out=x_sb, in_=x4_d[:].rearrange("k p s -> p k s"))
            wq_sb = const.tile([128, 3, 4, 128], BF16)
            wk_sb = const.tile([128, 3, 4, 128], BF16)
            nc.sync.dma_start(out=wk_sb, in_=wkT_d[:].rearrange("i k p f -> p i k f"))
            nc.sync.dma_start(out=wq_sb, in_=wqT_d[:].rearrange("i k p f -> p i k f"))
            othp = ctx.enter_context(tc.tile_pool(name="othp", bufs=1))
            oth1 = othp.tile([128, 4, S], BF16, name="oth")
            nc.sync.dma_start(out=oth1, in_=oth_d[0].rearrange("k p s -> p k s"))
            wvo_sb = const.tile([128, 4, 384], BF16)
            nc.sync.dma_start(out=wvo_sb, in_=wvoT_d[:].rearrange("k p f -> p k f"))

            # Small consts: DMA to staging, then re-own on the consuming
            # engine (DVE / ACT) so consumers need no cross-engine const wait.
            dvec_st = const.tile([128, 6], F32)
            nc.gpsimd.dma_start(out=dvec_st, in_=dvec_d[:])
            wobv_st = const.tile([128, 384], F32)
            nc.gpsimd.dma_start(out=wobv_st, in_=wobv_d[:].partition_broadcast(128))
            avec_st = const.tile([128, 10], F32)
            nc.gpsimd.dma_start(out=avec_st, in_=avec_d[:])
            dvec = const.tile([128, 6], F32)
            nc.vector.tensor_copy(dvec, dvec_st)
            wobv_sb = const.tile([128, 384], F32)
            nc.vector.tensor_copy(wobv_sb, wobv_st)
            avec = const.tile([128, 10], F32)
            nc.scalar.activation(out=avec, in_=avec_st, func=AF.Copy)
            bqv_sb = dvec[:, 0:3]
            bkv_sb = dvec[:, 3:6]

            # xt (attention output) slabs + h1 slabs, zero-padded 34x34
            xt_sl = []
            for i in range(3):
                t = xtp.tile([128, HP, HP], BF16, name=f"xt{i}")
                nc.vector.memset(t, 0.0)
                xt_sl.append(t)
            h1_sl = []
            for k in range(4):
                t = xtp.tile([128, HP, HP], BF16, name=f"h1{k}")
                nc.vector.memset(t, 0.0)
                h1_sl.append(t)
            # per-branch conv1 partials (f32 staging for the AllReduce);
            # 2 rotating slots — branch i is shipped before i+1 is written
            brst = {}

            def get_brst(i):
                if i not in brst:
                    brst[i] = brp.tile([128, 4, S], BF16, name="brst")
                return brst[i]

            # Semaphore warmers: absorb const-DMA + memset waits into each
            # engine's observed clock so later compute ops need <=1 wait.
            warm = const.tile([128, 1], F32)
            nc.vector.tensor_copy(warm, dvec[:, 0:1])
            warm2 = const.tile([128, 1], F32)
            nc.scalar.activation(out=warm2, in_=warm, func=AF.Copy)

            # k/q per branch (with biases added), uT tiles
            k_sb = kq.tile([128, 3, S], BF16)
            q_sb = kq.tile([128, 3, S], BF16)
            uT = [kq.tile([128, 3, 4, 33], BF16, name=f"uT{t}") for t in range(8)]

            # ---- projections (k0/q0 upfront; rest fill branch-0 slots) ---
            # x_prev reuses x_next's slot once q_proj(0) has consumed it
            qsrc = [oth1, x_sb, None]

            def load_oth2():
                oth2 = othp.tile([128, 4, S], BF16, name="oth")
                nc.sync.dma_start(out=oth2, in_=oth_d[1].rearrange("k p s -> p k s"))
                qsrc[2] = oth2

            def k_proj(i):
                k_ps = scps.tile([128, S], F32, name="kq_ps", tag="sc")
                for s in range(2):
                    for ks in range(4):
                        nc.tensor.matmul(
                            k_ps[:, 512 * s : 512 * (s + 1)],
                            lhsT=wk_sb[:, i, ks, :],
                            rhs=x_sb[:, ks, 512 * s : 512 * (s + 1)],
                            start=(ks == 0),
                            stop=(ks == 3),
                        )
                nc.vector.tensor_scalar_add(k_sb[:, i, :], k_ps, bkv_sb[:, i : i + 1])

            def q_proj(i):
                q_ps = scps.tile([128, S], F32, name="kq_ps2", tag="sc")
                for s in range(2):
                    for ks in range(4):
                        nc.tensor.matmul(
                            q_ps[:, 512 * s : 512 * (s + 1)],
                            lhsT=wq_sb[:, i, ks, :],
                            rhs=qsrc[i][:, ks, 512 * s : 512 * (s + 1)],
                            start=(ks == 0),
                            stop=(ks == 3),
                        )
                nc.vector.tensor_scalar_add(q_sb[:, i, :], q_ps, bqv_sb[:, i : i + 1])

            def u_proj(t):
                u_ps = smps.tile([128, 384], F32, name="u_ps", tag="sm")
                for ks in range(4):
                    nc.tensor.matmul(
                        u_ps,
                        lhsT=x_sb[:, ks, 128 * t : 128 * (t + 1)],
                        rhs=wvo_sb[:, ks, :],
                        start=(ks == 0),
                        stop=(ks == 3),
                    )
                nc.vector.memset(uT[t][:, :, :, 32:33], 1.0)
                # wobv has the attention-BN bias folded in host-side
                nc.vector.tensor_add(
                    uT[t][:, :, :, 0:32],
                    u_ps.rearrange("p (i h d) -> p i h d", i=3, h=4),
                    wobv_sb.rearrange("p (i h d) -> p i h d", i=3, h=4),
                )

            k_proj(0)
            q_proj(0)
            load_oth2()

            # ---- conv weights (emitted after proj psum freed) ------------
            convw = ctx.enter_context(tc.tile_pool(name="convw", bufs=1))
            pt = ctx.enter_context(tc.tile_pool(name="pt", bufs=16))
            c1w_sb = [
                [convw.tile([128, 9, 128], BF16, name=f"c1w{i}_{m}") for m in range(4)]
                for i in range(3)
            ]
            for i in range(3):
                for m in range(4):
                    nc.sync.dma_start(out=c1w_sb[i][m], in_=c1wT_d[i, m])
            c2w_sb = [convw.tile([128, 9, 128], BF16, name=f"c2w{k}") for k in range(4)]
            for k in range(4):
                nc.sync.dma_start(out=c2w_sb[k], in_=c2wT_d[k])

            def conv1_block(i, m, n):
                """Partial conv1 for xt slab i, out m-tile, spatial half n,
                written (bf16) into brst[i]."""
                ps = smps.tile([128, 512], F32, name="cv", tag="sm")
                for dy in range(3):
                    for dx in range(3):
                        nc.tensor.matmul(
                            ps,
                            lhsT=c1w_sb[i][m][:, dy * 3 + dx, :],
                            rhs=xt_sl[i][:, 16 * n + dy : 16 * n + dy + 16, dx : dx + 32],
                            start=(dy == 0 and dx == 0),
                            stop=(dy == 2 and dx == 2),
                        )
                nc.vector.tensor_copy(get_brst(i)[:, m, 512 * n : 512 * (n + 1)], ps)
                if n == 1:
                    if i < 2:
                        dst = partial[i][128 * m : 128 * (m + 1), :]
                    else:
                        dst = partial2[m // 2][128 * (m % 2) : 128 * (m % 2) + 128, :]
                    nc.gpsimd.dma_start(out=dst, in_=get_brst(i)[:, m, :])

            def attention(i, pr, filler):
                """Heads (2pr, 2pr+1) of branch i.  `filler` is a list of
                thunks (conv1 blocks / projections) sprinkled between the
                per-t score groups to keep PE dense while ACT grinds exps."""
                heads = (2 * pr, 2 * pr + 1)
                pts = {}
                fi = 0
                for t in range(8):
                    for h in heads:
                        sc = scps.tile([128, S], F32, name="sc", tag="sc")
                        p0 = 32 * h
                        for s in range(2):
                            nc.tensor.matmul(
                                sc[:, 512 * s : 512 * (s + 1)],
                                lhsT=k_sb[p0 : p0 + 32, i, 128 * t : 128 * (t + 1)],
                                rhs=q_sb[p0 : p0 + 32, i, 512 * s : 512 * (s + 1)],
                                start=True,
                                stop=True,
                                tile_position=(p0, 0),
                            )
                        ptt = pt.tile([128, S], BF16, name="ptt")
                        nc.scalar.activation(
                            out=ptt, in_=sc, func=AF.Exp, scale=float(ISQD)
                        )
                        pts[(h, t)] = ptt
                    while fi < len(filler) * (t + 1) // 8:
                        filler[fi]()
                        fi += 1
                # y chains: per head, the two query-half chains target the
                # two zero regions of one [33, S] psum tile sequentially
                for h in heads:
                    y2 = yps.tile([33, S], F32, name="y2", tag="y")
                    for s in range(2):
                        for t in range(8):
                            nc.tensor.matmul(
                                y2[:, 512 * s : 512 * (s + 1)],
                                lhsT=uT[t][:, i, h, :],
                                rhs=pts[(h, t)][:, 512 * s : 512 * (s + 1)],
                                start=(t == 0),
                                stop=(t == 7),
                            )
                    p0 = 32 * h
                    # 1/denom as exp(-ln(denom)) on ACT: ln and exp share the
                    # natural_log_exp_and_others table set (no reload), and
                    # both DVE reciprocal (6.5ns/elem) and the custom-DVE
                    # approx op (garbage on this HW) are avoided.
                    rc = rcp.tile([1, S], F32, name="rc")
                    nc.scalar.activation(out=rc, in_=y2[32:33, :], func=AF.Ln)
                    nc.scalar.activation(out=rc, in_=rc, func=AF.Exp, scale=-1.0)
                    rcb = rcp.tile([32, S], F32, name="rcb")
                    nc.gpsimd.partition_broadcast(rcb, rc[:])
                    nc.vector.tensor_mul(
                        xt_sl[i][p0 : p0 + 32, 1:33, 1:33],
                        y2[0:32, :].rearrange("p (a b) -> p a b", b=32),
                        rcb.rearrange("p (a b) -> p a b", b=32),
                    )
                while fi < len(filler):
                    filler[fi]()
                    fi += 1

            # ---- collectives ---------------------------------------------
            # branch 0/1: one [512, S] bf16 AllReduce each, overlapped with
            # the next branch's attention.  branch 2: two [256, S] chunks
            # (m01 / m23) pipelined against h1+conv2.
            partial = [dram.tile([512, S], BF16, name=f"part{i}") for i in range(2)]
            art = [dram.tile([512, S], BF16, name=f"art{i}") for i in range(2)]
            partial2 = [dram.tile([256, S], BF16, name=f"part2{a}") for a in range(2)]
            art2 = [dram.tile([256, S], BF16, name=f"art2{a}") for a in range(2)]

            def ar_branch(i):
                nc.gpsimd.collective_compute(
                    "AllReduce",
                    mybir.AluOpType.add,
                    replica_groups=GROUPS,
                    ins=[partial[i][:]],
                    outs=[art[i][:]],
                )

            def ar2_chunk(a):
                nc.gpsimd.collective_compute(
                    "AllReduce",
                    mybir.AluOpType.add,
                    replica_groups=GROUPS,
                    ins=[partial2[a][:]],
                    outs=[art2[a][:]],
                )

            def conv1_and_ship(i):
                return [
                    (lambda m=m, n=n: conv1_block(i, m, n))
                    for m in range(4)
                    for n in range(2)
                ]

            # ---- phase A: attention with projections/conv1 interleaved ---
            attention(0, 0, [lambda t=t: u_proj(t) for t in range(8)])
            attention(0, 1, [lambda: k_proj(1), lambda: q_proj(1),
                             lambda: k_proj(2), lambda: q_proj(2)])
            f0 = conv1_and_ship(0)
            attention(1, 0, f0[:4])
            attention(1, 1, f0[4:] + [lambda: ar_branch(0)])
            f1 = conv1_and_ship(1)
            attention(2, 0, f1[:4])
            attention(2, 1, f1[4:] + [lambda: ar_branch(1)])
            # branch 2 conv1 + chunked AR at the end
            for m in range(4):
                for n in range(2):
                    conv1_block(2, m, n)
                if m == 1:
                    ar2_chunk(0)
            ar2_chunk(1)

            # ---- phase C: combine, BN1+relu, conv2, BN2+relu, out --------
            arr01 = stg.tile([128, 4, S], BF16, name="arr01", bufs=1)
            arrt = [stg.tile([128, 2, S], BF16, name=f"arrt{j}", bufs=1) for j in range(2)]
            # art0 + art1 (during late attention / AR2 flight)
            nc.sync.dma_start(
                out=arr01, in_=art[0][:].rearrange("(m p) s -> p m s", p=128)
            )
            nc.sync.dma_start(
                out=arrt[0],
                in_=art[1][:].rearrange("(m p) s -> p m s", p=128)[:, 0:2, :],
            )
            nc.sync.dma_start(
                out=arrt[1],
                in_=art[1][:].rearrange("(m p) s -> p m s", p=128)[:, 2:4, :],
            )
            nc.vector.tensor_add(
                arr01[:, 0:2, :], arr01[:, 0:2, :], arrt[0][:, 0:2, :]
            )
            nc.vector.tensor_add(
                arr01[:, 2:4, :], arr01[:, 2:4, :], arrt[1][:, 0:2, :]
            )

            oout = stg.tile([128, S], F32, name="oout", bufs=1)
            ps2 = [smps.tile([128, 512], F32, name=f"cv2_{n}", tag="sm") for n in range(2)]

            def h1_chunk(a):
                """arr01[m01/m23] + art2 chunk -> BN1+relu -> h1 slabs."""
                nc.sync.dma_start(
                    out=arrt[0],
                    in_=art2[a][:].rearrange("(m p) s -> p m s", p=128),
                )
                src = arrt[0]
                nc.vector.tensor_add(
                    src[:, 0:2, :], src[:, 0:2, :], arr01[:, 2 * a : 2 * a + 2, :]
                )
                for j, k in enumerate((2 * a, 2 * a + 1)):
                    nc.scalar.activation(
                        out=h1_sl[k][:, 1:33, 1:33],
                        in_=src[:, j, :].rearrange("p (a b) -> p a b", b=32),
                        func=AF.Relu,
                        bias=avec[:, 4 + k : 5 + k],
                        scale=avec[:, k : k + 1],
                    )

            def conv2_half(a):
                # accumulate k-slabs 2a, 2a+1 into both spatial halves
                for n in range(2):
                    for k in (2 * a, 2 * a + 1):
                        for dy in range(3):
                            for dx in range(3):
                                nc.tensor.matmul(
                                    ps2[n],
                                    lhsT=c2w_sb[k][:, dy * 3 + dx, :],
                                    rhs=h1_sl[k][
                                        :, 16 * n + dy : 16 * n + dy + 16, dx : dx + 32
                                    ],
                                    start=(k == 0 and dy == 0 and dx == 0),
                                    stop=(k == 3 and dy == 2 and dx == 2),
                                )

            h1_chunk(0)
            conv2_half(0)   # overlaps AR2 chunk 1
            h1_chunk(1)
            conv2_half(1)
            for n in range(2):
                nc.scalar.activation(
                    out=oout[:, 512 * n : 512 * (n + 1)],
                    in_=ps2[n],
                    func=AF.Relu,
                    bias=avec[:, 9:10],
                    scale=avec[:, 8:9],
                )
                nc.sync.dma_start(
                    out=out_d[:, 512 * n : 512 * (n + 1)],
                    in_=oout[:, 512 * n : 512 * (n + 1)],
                )

    nc.finalize()
    return nc


def _f(x):
    return np.ascontiguousarray(x, dtype=np.float32)


def _bf(x):
    return np.ascontiguousarray(np.asarray(x, dtype=np.float32).astype(ml_dtypes.bfloat16))


def prepare_core_inputs(inp):
    """Build the 8 per-core input dicts from the full-problem inputs."""
    inp = {k: np.asarray(v, dtype=np.float64) for k, v in inp.items()}
    x = inp["x"].reshape(B, C, S)
    xp = inp["x_prev"].reshape(B, C, S)
    xn = inp["x_next"].reshape(B, C, S)

    bn1s_full = inp["bn1g"] / np.sqrt(inp["bn1v"] + EPS)
    bn1b_full = inp["bn1b"] - inp["bn1m"] * bn1s_full
    bn2s_full = inp["bn2g"] / np.sqrt(inp["bn2v"] + EPS)
    bn2b_full = inp["bn2b"] - inp["bn2m"] * bn2s_full

    per_g = []
    for g in range(4):
        sl = slice(128 * g, 128 * (g + 1))
        wqT = np.stack(
            [
                np.stack([inp["Wq"][i][sl, 128 * k : 128 * (k + 1)].T for k in range(4)])
                for i in range(3)
            ]
        )
        wkT = np.stack(
            [
                np.stack([inp["Wk"][i][sl, 128 * k : 128 * (k + 1)].T for k in range(4)])
                for i in range(3)
            ]
        )
        bqv = np.stack([inp["bq"][i][sl] for i in range(3)], axis=1)
        bkv = np.stack([inp["bk"][i][sl] for i in range(3)], axis=1)

        att_s = np.stack(
            [inp["bng"][i][sl] / np.sqrt(inp["bnv"][i][sl] + EPS) for i in range(3)]
        )  # (3,128)
        xtb = np.stack(
            [
                inp["bnb"][i][sl] + (inp["bo"][i][sl] - inp["bnm"][i][sl]) * att_s[i]
                for i in range(3)
            ]
        )  # (3,128)

        wvo_rows = []
        wobv_row = []
        for i in range(3):
            for hl in range(4):
                hg = 4 * g + hl
                wv_h = inp["Wv"][i][32 * hg : 32 * (hg + 1), :]  # (32, 512)
                bv_h = inp["bv"][i][32 * hg : 32 * (hg + 1)]
                wo_h = inp["Wo"][i, hg]  # (32, 32)
                sc = att_s[i][32 * hl : 32 * (hl + 1)]  # (32,)
                wvo_rows.append(sc[:, None] * (wo_h @ wv_h))
                # fold the (BN-scaled) output bias + BN bias into the u bias:
                # y/denom + xtb == sum_k (u + xtb) p_k / denom
                wobv_row.append(sc * (wo_h @ bv_h) + xtb[i][32 * hl : 32 * (hl + 1)])
        wvo_all = np.concatenate(wvo_rows, axis=0)  # (384, 512)
        wobv = np.concatenate(wobv_row)[None, :]  # (1, 384)
        wvoT = np.stack([wvo_all[:, 128 * k : 128 * (k + 1)].T for k in range(4)])

        c1wT = np.stack(
            [
                np.stack(
                    [
                        inp["c1w"][
                            128 * m : 128 * (m + 1),
                            512 * i + 128 * g : 512 * i + 128 * (g + 1),
                        ]
                        .transpose(1, 2, 3, 0)
                        .reshape(128, 9, 128)
                        for m in range(4)
                    ]
                )
                for i in range(3)
            ]
        )
        c2wT = np.stack(
            [
                inp["c2w"][sl, 128 * k : 128 * (k + 1)]
                .transpose(1, 2, 3, 0)
                .reshape(128, 9, 128)
                for k in range(4)
            ]
        )
        avec = np.concatenate(
            [
                bn1s_full.reshape(4, 128).T,
                bn1b_full.reshape(4, 128).T,
                bn2s_full[sl][:, None],
                bn2b_full[sl][:, None],
            ],
            axis=1,
        )  # (128, 10)

        per_g.append(
            dict(
                wqT=_bf(wqT), wkT=_bf(wkT), wvoT=_bf(wvoT),
                wobv=_f(wobv), c1wT=_bf(c1wT), c2wT=_bf(c2wT),
                dvec=_f(np.concatenate([bqv, bkv], axis=1)),
                avec=_f(avec),
            )
        )

    in_maps = []
    for c in range(NCORES):
        b, g = c // 4, c % 4
        d = dict(per_g[g])
        d["x4"] = _bf(x[b].reshape(4, 128, S))
        d["oth"] = _bf(np.stack([xn[b].reshape(4, 128, S), xp[b].reshape(4, 128, S)]))
        in_maps.append(d)
    return in_maps


_NC_CACHE = {}


def get_nc():
    if "nc" not in _NC_CACHE:
        _NC_CACHE["nc"] = build_nc()
    return _NC_CACHE["nc"]


def assemble(results):
    out = np.zeros((B, C, H, W), dtype=np.float32)
    for c in range(NCORES):
        b, g = c // 4, c % 4
        out[b, 128 * g : 128 * (g + 1)] = results[c]["out"].reshape(128, H, W)
    return out


def kernel(**inputs):
    nc = get_nc()
    in_maps = prepare_core_inputs(inputs)
    res = run_bass_kernel_spmd(nc, in_maps, list(range(NCORES)))
    return assemble(res.results)
